# revision 34
# baseline (speedup 1.0000x reference)
"""CPRLinearMultiPrecision kernel for 8 TRN2 NeuronCores — fp8 DoubleRow.

The reference absmax is dominated by the high-precision columns (out std
~55 vs ~6 for the low columns), so the 2e-2 relative-error budget leaves
the low columns ~5 absolute error headroom.  That pays for an fp8
matmul on the low 75% of columns:

  * low cols:  W8 = e4m3((q - z[k]) * s1[k] * s2[c]) prepared on host,
    x8 = e4m3(x).  PE runs perf_mode=DoubleRow: both operands
    [128, 2, N] fp8, contracting TWO 128-row K-groups per column-cycle
    (~1.13 cyc/col vs 1.0 for fp16 but half the matmuls).  Measured
    numpy error: max |err| ~2.0 « 6.5 budget (rel ~6e-3 total).
  * high cols: exact int8*scale structure needs >4 significant bits, and
    a 2-pass fp8 split only ties fp16, so they stay an fp16 matmul with
    host-dequantized W.

Every core gets an equal mix (column-parallel, rebalanced): 344 high
cols + 8 stray low cols as the fp16 path (352 = one PSUM chunk) + 1024
low cols as the fp8 path (2 x 512 chunks).  Per 128-token subtile the
PE runs 32 fp16 matmuls (N=352, 149ns) + 16 DoubleRow pairs (2 matmuls
of N=512, 216ns — no measured DR streaming penalty), ~11.9us; subtiles
run in pairs (fp16 A, fp16 B, DR A, DR B — six PSUM banks hold exactly
one pair) to halve the fp16<->DR phase transitions; the fp8 x tile is
cast on-device by the DVE rather than DMA'd, and the DVE also drains
psum+bias -> fp16 output tiles.

t=0 is supply-bound: all of W (7.1MB) plus 4MB of x must land at the
~345GB/s aggregate HBM cap before the PE has anything to chew on, and
the HAM clock state machine makes this expensive twice over — the PE
boots at K=4/8 (half clock), upshifts only after ~3.2us of GAPLESS
matmul activity, and downshifts again on any ~2us idle.  Countermeasures
(worth ~7us at 2.4GHz vs the plain EDF prologue):
  * 32 dependency-free garbage matmuls right after the preamble warm the
    array to K=8/8 by ~13us and bridge the PE to the first real supply
    (the rings only start flowing at ~9.1/10.6/12.3us and share the cap
    roughly 85/105/160 GB/s once all active);
  * the first pair runs pair-chunk-major (A then B per 4-group w16
    chunk, x in NG-quarters; per-chunk stalls padded with 1-2 garbage
    matmuls stay under the downshift threshold), then chunk-major DR;
  * the second pair (C,D) is m-tile-major since x03 lands ~8us after
    x02; steady-state pairs then run exactly at the 23.36us streaming
    floor (measured 23.38).
The final subtile's last 512-col chain is evicted in 4x128-col slices
across both HWDGE rings so only ~1.3us of add+store rides the tail.

Host side: dequantize/quantize W, fold s1/s2/z into the fp8 codes (bias
is applied on device); gather = concat + column permutation.

Measured on 8 TRN2 cores: ~409us HW exec at 2.4GHz (vs 411.3us for the
plain-EDF baseline; PE streaming floor 373.8us + ~8us preamble+ramp +
~5us counted tail), rel err ~5.5e-3 vs the fp32 reference (budget
2e-2).  Beware run-to-run P0 downclock (PE upshifts to 2.0GHz instead
of 2.4): check MATMUL start-to-start is 149/216ns before comparing
timings.

Post-scheduling passes (unchanged from the fp16 baseline):
_dedupe_ldweights drops back-to-back LDWEIGHTS reloading an unchanged
stationary tile (the compiler-side --enable-ldw-opt is disabled);
_reduce_waits prunes transitively-implied semaphore waits; and
_legalize_waits moves surplus waits onto same-engine Drains (engine ISA
structs encode a single sync-wait slot).
"""

import numpy as np
import ml_dtypes
from contextlib import ExitStack

import concourse.bass as bass
import concourse.tile as tile
from concourse import mybir
from concourse.bass_utils import run_bass_kernel_spmd

# Problem shapes (hardcoded; kernel.py must be self-contained).
B, S = 2, 2048
M = B * S              # 4096 tokens
K = 4096               # in features
OUT_F = 11008
N_HIGH = 2752
N_LOW = OUT_F - N_HIGH  # 8256
GROUP = 128
NG = K // GROUP        # 32 K-groups
NPAIR = NG // 2        # 16 DoubleRow group-pairs
NCORES = 8
NH = N_HIGH // NCORES  # 344 high cols per core
NL = N_LOW // NCORES   # 1032 low cols per core
NW16 = NH + 8          # fp16-path cols per core (344 high + 8 stray low)
NW8 = NL - 8           # fp8-path cols per core (1024)
NSH = NW16 + NW8       # 1376 output cols per core
P = 128
TOK_TILE = 512
NT = M // TOK_TILE     # 8
GCHUNK = 4             # fp16 W groups per DMA chunk
PCHUNK = 2             # fp8 W pairs per DMA chunk

f8 = mybir.dt.float8e4
f16 = mybir.dt.float16
f32 = mybir.dt.float32

DR = mybir.MatmulPerfMode.DoubleRow


def _bcast(ap_1d, parts):
    """Partition-broadcast AP: replicate a 1-D DRAM row across `parts` partitions."""
    return bass.AP(tensor=ap_1d.tensor, offset=ap_1d.offset, ap=[[0, parts]] + ap_1d.ap)


def _kernel_body(ctx, tc, out, xt, w16, wf8, biasv):
    nc = tc.nc
    const = ctx.enter_context(tc.tile_pool(name="const", bufs=1))
    w16pool = ctx.enter_context(tc.tile_pool(name="w16", bufs=NG // GCHUNK))
    w8pool = ctx.enter_context(tc.tile_pool(name="w8", bufs=NPAIR // PCHUNK))
    xpool = ctx.enter_context(tc.tile_pool(name="x", bufs=8))
    xqpool = ctx.enter_context(tc.tile_pool(name="xq", bufs=8))
    x8pool = ctx.enter_context(tc.tile_pool(name="x8", bufs=8))
    opool = ctx.enter_context(tc.tile_pool(name="o", bufs=4))
    ppool = ctx.enter_context(tc.tile_pool(name="p", bufs=6, space="PSUM"))

    # t=0 needs ALL of W (7.1MB) plus its tt=0 x slices within its first
    # ~12us of PE work, and each DMA ring sustains only ~125GB/s draining
    # FIFO — so x is tiled per 128-token subtile (1.5MB/subtile instead of
    # 6.3MB/t upfront) and the t0-critical transfers are spread across all
    # three rings (sync/scalar HWDGE + gpsimd SWDGE) in PE consumption
    # order (earliest-deadline-first).
    w16t = [w16pool.tile([P, GCHUNK, NW16], f16, tag="w16", name=f"w16c{i}")
            for i in range(NG // GCHUNK)]
    w8t = [w8pool.tile([P, PCHUNK, 2, NW8], f8, tag="w8", name=f"w8c{i}")
           for i in range(NPAIR // PCHUNK)]
    garb = const.tile([P, 512], f16)
    bias_b = const.tile([P, NSH], f32)

    def load_x_tt(t, tt, eng=None):
        xc = xpool.tile([P, NG, P], f16, tag="xc")
        (eng or nc.sync).dma_start(out=xc[:], in_=xt[t, tt, :, :, :])
        return xc

    def cast_x8(xc):
        # fp8 copy of the x subtile for the DoubleRow path — derived
        # on-device (DVE) instead of a second HBM stream, keeping the
        # ring-bound t0 prologue lean.
        x8c = x8pool.tile([P, NG, P], f8, tag="x8c")
        nc.vector.tensor_copy(x8c[:], xc[:])
        return x8c

    # --- PE clock warm-up -------------------------------------------------
    # The HAM state machine starts the PE at K=4/8 (half clock) and only
    # upshifts after ~3.2us of GAPLESS matmul activity; any >=0.5us idle
    # resets the accumulator, and t0 is full of supply stalls — on the
    # baseline the array stayed at half clock until t~42us.  A burst of
    # dependency-free garbage matmuls right after the framework preamble
    # (PE idle until ~18us otherwise: DMA rings only start flowing at
    # ~9.6us) upshifts the clock by ~12us, before the first real matmul.
    nc.vector.memset(garb[:], 1.0)
    dummy_ps = ppool.tile([P, 512], f32, tag="ps", name="warm")

    def dummies(n):
        for _ in range(n):
            nc.tensor.matmul(dummy_ps[:, :], garb[:, :P], garb[:, :],
                             start=True, stop=True)

    dummies(32)

    # Prologue, spread EDF-style over the three rings in PE consumption
    # order.  The fp16 phase of the first pair runs pair-chunk-major, so
    # its critical supply (w16c0 + x(0,0)/x(0,1) in NG-quarters) leads the
    # fast-starting sync ring; w16c1..7 ride scalar; wf8 is split so the
    # chunk-major DR(A,B) phase never waits:
    #   sync:   w16c0, x00/x01 quarters (interleaved), wf8c5-6, x02, x03
    #   scalar: w16c1..4, bias, w16c5..7
    #   gpsimd: wf8c0..4, wf8c7
    def load_w8(i, eng):
        eng.dma_start(out=w8t[i][:], in_=wf8[:, i * PCHUNK:(i + 1) * PCHUNK, :, :])

    # EDF split across the three rings using their measured shares when all
    # are active (sync ~85GB/s, scalar ~105, gpsimd-SWDGE ~160; the SWDGE
    # ring starts ~3.5us later but drains fastest).  Deadline order is the
    # PE's: w16c+x-quarters paced ~1.4us/chunk from ~15us, then wf8 chunks
    # ~2us apart from ~29us, then x02/x03/bias for the C,D pair.
    def load_w16(i, eng):
        eng.dma_start(out=w16t[i][:], in_=w16[:, i * GCHUNK:(i + 1) * GCHUNK, :])

    def load_xq(tt, j, eng):
        q = xqpool.tile([P, 8, P], f16, tag="xq")
        eng.dma_start(out=q[:], in_=xt[0, tt, :, 8 * j:8 * j + 8, :])
        return q

    xq = [[None] * 4 for _ in range(2)]  # xq[tt][j]: groups 8j..8j+7 of x(0,tt)
    # sync: first-matmul critical path (w16c0 + all x quarters), then the
    # late wf8 chunks and x03
    load_w16(0, nc.sync)
    for j in range(4):
        xq[0][j] = load_xq(0, j, nc.sync)
        xq[1][j] = load_xq(1, j, nc.sync)
    for i in (5, 6):
        load_w8(i, nc.sync)
    x02 = load_x_tt(0, 2, nc.sync)
    x03 = load_x_tt(0, 3, nc.sync)
    # scalar: the rest of w16, bias mid-stream
    for i in (1, 2, 3, 4):
        load_w16(i, nc.scalar)
    nc.scalar.dma_start(out=bias_b[:], in_=_bcast(biasv[:], P))
    for i in (5, 6, 7):
        load_w16(i, nc.scalar)
    # gpsimd (fast once its ~3.5us SWDGE spin-up passes): the early wf8
    # chunks (consumed from ~28us).  NOTE: shifting MORE onto the SWDGE
    # ring (all 8 wf8 chunks, or w16/x tiles) measured consistently worse
    # — its effective rate degrades with queue depth, and the HWDGE/SWDGE
    # share split drifts ±30% run-to-run, so this EDF split was tuned
    # empirically, not from a rate model.
    for i in (0, 1, 2, 3, 4, 7):
        load_w8(i, nc.gpsimd)
    cur23 = [x02, x03]

    def fp16_chain(xc, ps0):
        # stationary x [128k x 128tok], moving W16 [128k x 352]
        for g in range(NG):
            nc.tensor.matmul(
                ps0[:, :], xc[:, g, :],
                w16t[g // GCHUNK][:, g % GCHUNK, :],
                start=(g == 0), stop=(g == NG - 1),
            )

    def dr_chains(x8c, ps1, ps2, order):
        # fp8 DoubleRow: both operands [128, 2, free]; contracts groups
        # (2p, 2p+1) per column-cycle.
        for p, ci in order:
            c0, pst = ((0, ps1), (512, ps2))[ci]
            nc.tensor.matmul(
                pst[:, :], x8c[:, 2 * p:2 * p + 2, :],
                w8t[p // PCHUNK][:, p % PCHUNK, :, c0:c0 + 512],
                start=(p == 0), stop=(p == NPAIR - 1),
                perf_mode=DR,
            )

    def evict(t, tt, ps, last):
        row0 = t * TOK_TILE + tt * P
        osb = opool.tile([P, NSH], f16, tag="osb")
        if not last:
            for c0, cw, pst in ((0, NW16, ps[0]), (NW16, 512, ps[1]),
                                (NW16 + 512, 512, ps[2])):
                nc.vector.tensor_add(osb[:, c0:c0 + cw], pst[:],
                                     bias_b[:, c0:c0 + cw])
            nc.gpsimd.dma_start(out=out[row0:row0 + P, :], in_=osb[:])
            return
        # Final subtile: first two chunks drain while the sliced last DR
        # chain still streams; the 512-col tail goes out in 4x128 slices
        # alternating across both HWDGE rings so only ~1us of add+store
        # trails the last matmul.
        for c0, cw, pst in ((0, NW16, ps[0]), (NW16, 512, ps[1])):
            nc.vector.tensor_add(osb[:, c0:c0 + cw], pst[:], bias_b[:, c0:c0 + cw])
            nc.scalar.dma_start(out=out[row0:row0 + P, c0:c0 + cw],
                                in_=osb[:, c0:c0 + cw])
        for s in range(4):
            c0 = NW16 + 512 + 128 * s
            nc.vector.tensor_add(osb[:, c0:c0 + 128],
                                 ps[2][:, 128 * s:128 * s + 128],
                                 bias_b[:, c0:c0 + 128])
            eng = (nc.sync, nc.scalar)[s % 2]
            eng.dma_start(out=out[row0:row0 + P, c0:c0 + 128],
                          in_=osb[:, c0:c0 + 128])

    def alloc_ps():
        return [ppool.tile([P, NW16], f32, tag="ps", name="ps0"),
                ppool.tile([P, 512], f32, tag="ps", name="ps1"),
                ppool.tile([P, 512], f32, tag="ps", name="ps2")]

    # --- t0 pair (A,B): supply-paced chunk-major ------------------------
    # The first pair is DMA-supply-bound (all of W plus 4MB of x must
    # stream in at ~370GB/s aggregate while the PE wants to run).  Both
    # fp16 chains run pair-chunk-major (A then B per 4-group w16 chunk, x
    # in NG-quarters) and both DR chains pair-chunk-major per wf8 chunk,
    # so each arriving chunk unlocks ~1.2-1.7us of work and no single
    # stall exceeds the ~3us HAM downshift threshold.
    def cast_x8_q(qs):
        x8c = x8pool.tile([P, NG, P], f8, tag="x8c")
        for j in range(4):
            nc.vector.tensor_copy(x8c[:, 8 * j:8 * j + 8, :], qs[j][:])
        return x8c

    x8A, x8B = cast_x8_q(xq[0]), cast_x8_q(xq[1])
    psA, psB = alloc_ps(), alloc_ps()
    for c in range(NG // GCHUNK):
        for tt in (0, 1):
            ps0 = (psA, psB)[tt][0]
            for g4 in range(GCHUNK):
                g = GCHUNK * c + g4
                j = g // 8
                nc.tensor.matmul(
                    ps0[:, :], xq[tt][j][:, g - 8 * j, :],
                    w16t[c][:, g4, :],
                    start=(g == 0), stop=(g == NG - 1),
                )
        if c < NG // GCHUNK - 1:
            # keep the supply-paced gap under the ~1.9us HAM downshift
            # threshold while the next w16 chunk streams in (the scalar
            # ring's ~3.4us/chunk cadence leaves ~2us holes from c3 on)
            dummies(1 if c < 3 else 2)


    for pc in range(NPAIR // PCHUNK):
        for tt in (0, 1):
            x8c = (x8A, x8B)[tt]
            ps1, ps2 = (psA, psB)[tt][1], (psA, psB)[tt][2]
            for pp in range(PCHUNK * pc, PCHUNK * (pc + 1)):
                for c0, pst in ((0, ps1), (512, ps2)):
                    nc.tensor.matmul(
                        pst[:, :], x8c[:, 2 * pp:2 * pp + 2, :],
                        w8t[pc][:, pp % PCHUNK, :, c0:c0 + 512],
                        start=(pp == 0), stop=(pp == NPAIR - 1),
                        perf_mode=DR,
                    )
    evict(0, 0, psA, False)
    evict(0, 1, psB, False)

    # --- t0 pair (C,D): m-tile-major ------------------------------------
    # x02/x03 land late (~38/46us) on the saturated rings, so C runs to
    # completion (its x tile arrives first) before D touches x03; the two
    # extra fp16<->DR transitions cost ~64ns each.
    x8C = cast_x8(cur23[0])
    psC = alloc_ps()
    fp16_chain(cur23[0], psC[0])
    nxt = [load_x_tt(1, s) for s in range(TOK_TILE // P)]
    dr_chains(x8C, psC[1], psC[2],
              [(p, ci) for p in range(NPAIR) for ci in (0, 1)])
    evict(0, 2, psC, False)
    x8D = cast_x8(cur23[1])
    psD = alloc_ps()
    fp16_chain(cur23[1], psD[0])
    dr_chains(x8D, psD[1], psD[2],
              [(p, ci) for p in range(NPAIR) for ci in (0, 1)])
    evict(0, 3, psD, False)

    # --- t=1..7: steady-state pairs -------------------------------------
    # Subtiles run in PAIRS — fp16(A), fp16(B), DR(A), DR(B).  Six PSUM
    # banks hold exactly one pair's three chains x2.
    cur = nxt
    for t in range(1, NT):
        for pr in range(2):
            ttA, ttB = 2 * pr, 2 * pr + 1
            # Prefetch next t's x tiles during the first pair.
            if pr == 0 and t + 1 < NT:
                nxt = [load_x_tt(t + 1, s) for s in range(TOK_TILE // P)]
            x8A, x8B = cast_x8(cur[ttA]), cast_x8(cur[ttB])
            psA, psB = alloc_ps(), alloc_ps()
            last = (t == NT - 1) and (pr == 1)
            fp16_chain(cur[ttA], psA[0])
            fp16_chain(cur[ttB], psB[0])
            dr_chains(x8A, psA[1], psA[2],
                      [(p, ci) for p in range(NPAIR) for ci in (0, 1)])
            evict(t, ttA, psA, False)
            if not last:
                dr_chains(x8B, psB[1], psB[2],
                          [(p, ci) for p in range(NPAIR) for ci in (0, 1)])
            else:
                # Final subtile: sequential DR chains so ps1 drains through
                # DVE/DMA while ps2 still streams — shortens the tail.
                # (Column-slicing the ps2 CHAIN is unsound: interleaved
                # accumulation chains in one PSUM bank clobber each other's
                # partials via the start_tensor_calc bank clear; only the
                # EVICTION is sliced, in evict().)
                dr_chains(x8B, psB[1], psB[2], [(p, 0) for p in range(NPAIR)])
                dr_chains(x8B, psB[1], psB[2], [(p, 1) for p in range(NPAIR)])
            evict(t, ttB, psB, last)
        cur = nxt if t + 1 < NT else None


# Engine-compute ISA structs encode very few sync-wait slots (the DVE
# tensor ops hold only one); walrus codegen hard-fails on excess.  Tile's
# scheduler may attach several waits to one instruction, so after
# scheduling we move the surplus onto same-engine Drain instructions
# inserted immediately before (the engine stalls there instead — same
# semantics, and drains legally carry many waits).
_WAIT_LIMITED = {
    "InstTensorTensor", "InstTensorScalarPtr", "InstTensorCopy",
    "InstTensorReduce", "InstMemset", "InstActivation", "InstIota",
    "InstMatmult", "InstLdweights", "InstBNStats", "InstBNStatsAggregate",
    "InstDrain", "InstDMACopy",
}


def _dedupe_ldweights(nc):
    """Delete back-to-back redundant LDWEIGHTS.

    The two column-chunk matmuls of each DoubleRow pair share one
    stationary tile, but bass emits an Ldweights per matmul and the
    compiler-side dedup (--enable-ldw-opt) is disabled.  Reloading
    identical weights is idempotent, so an Ldweights whose source AP
    equals the previous one on the PE stream — with only matmuls in
    between, no sync waits and no sem updates of its own — can be
    dropped.  W tiles are written once and never recycled, and x-tile
    slot reuse is gated on the matmuls' sem increments (Ldweights never
    increments), so sem bookkeeping is unchanged.
    """
    removed = 0
    for fn in nc.m.functions:
        for bb in fn.blocks:
            newl = []
            prev_ldw_key = None
            for inst in bb.instructions:
                t = type(inst).__name__
                eng = str(inst.engine)
                if eng == "EngineType.PE":
                    if t == "InstLdweights":
                        si = inst.sync_info
                        has_sync = si is not None and (si.on_wait or si.on_update)
                        key = str(inst.ins)
                        if key == prev_ldw_key and not has_sync:
                            removed += 1
                            continue
                        prev_ldw_key = key
                    elif t != "InstMatmult":
                        prev_ldw_key = None
                newl.append(inst)
            bb.instructions[:] = newl
    return removed


def _reduce_waits(nc):
    """Drop transitively-implied semaphore waits.

    A wait (sem, v) on instruction X is redundant when another wait on X
    targets a producer whose vector clock already covers (sem, v), when
    X's own proc has already observed it, or when the sem belongs to X's
    own in-order proc (same-FIFO dominance).  Two phases: build complete
    per-sem producer vector clocks (block list order is per-proc
    consistent; cross-proc misses only make the result conservative),
    then prune using the final maps.  Only 'sem-ge-imm' waits and
    incrementing ('sem-inc'/'sem-add-imm') updates participate; any other
    update mode invalidates that sem's history.
    """
    INC = ("sem-inc", "sem-add-imm")
    for fn in nc.m.functions:
        insts = [inst for bb in fn.blocks for inst in bb.instructions]

        def params(inst):
            si = inst.sync_info
            waits = list(si.on_wait) if si is not None and si.on_wait else []
            ups = list(si.on_update) if si is not None and si.on_update else []
            proc = (str(inst.engine), getattr(inst, "bass_scheduled_proc", None))
            return si, waits, ups, proc

        def wait_vc(prodvc, w):
            if w.wait_mode != "sem-ge-imm" or w.wait_reg is not None:
                return None
            for cv, vc in prodvc.get(w.id, []):
                if cv >= w.wait_value:
                    return vc
            return None

        def build_maps(lookup_prodvc):
            cum, prodvc, procvc, updaters, obsvc = {}, {}, {}, {}, {}
            for inst in insts:
                si, waits, ups, proc = params(inst)
                myvc = dict(procvc.get(proc, {}))
                for w in waits:
                    if w.wait_mode == "sem-ge-imm" and w.wait_reg is None:
                        vc = wait_vc(lookup_prodvc if lookup_prodvc is not None
                                     else prodvc, w)
                        if vc is not None:
                            for k, v in vc.items():
                                if myvc.get(k, 0) < v:
                                    myvc[k] = v
                        if myvc.get(w.id, 0) < w.wait_value:
                            myvc[w.id] = w.wait_value
                procvc[proc] = myvc
                obsvc[id(inst)] = myvc
                for u in ups:
                    if u.update_mode in INC and u.update_reg is None:
                        cum[u.id] = cum.get(u.id, 0) + u.update_value
                        updaters.setdefault(u.id, set()).add(proc)
                        snap = dict(myvc)
                        snap[u.id] = cum[u.id]
                        prodvc.setdefault(u.id, []).append((cum[u.id], snap))
                    else:
                        cum.pop(u.id, None)
                        prodvc.pop(u.id, None)
                        updaters[u.id] = {object()}
            return cum, prodvc, updaters, obsvc

        # Pass 1 builds conservative clocks; pass 2 rebuilds them resolving
        # forward references through pass 1's complete producer map.
        _, prodvc, _, _ = build_maps(None)
        _, prodvc, _, _ = build_maps(prodvc)

        # Prune with the final maps, tracking per-proc observation and
        # per-proc cumulative sem updates in list order.
        cum, procvc, updaters = {}, {}, {}
        for inst in insts:
            si, waits, ups, proc = params(inst)
            myvc = dict(procvc.get(proc, {}))
            if len(waits) > 1:
                vcs = [wait_vc(prodvc, w) for w in waits]
                keep_ws = []
                for i, w in enumerate(waits):
                    if w.wait_mode == "sem-ge-imm" and w.wait_reg is None:
                        if myvc.get(w.id, 0) >= w.wait_value:
                            continue
                        if (updaters.get(w.id) == {proc}
                                and cum.get(w.id, 0) >= w.wait_value):
                            continue
                        implied = any(
                            j != i and vcs[j] is not None
                            and vcs[j].get(w.id, 0) >= w.wait_value
                            for j in range(len(waits)))
                        if implied:
                            continue
                    keep_ws.append(w)
                if len(keep_ws) != len(waits):
                    inst.sync_info = mybir.SyncInfo(on_wait=keep_ws, on_update=ups)
                    waits = keep_ws
            for w in waits:
                if w.wait_mode == "sem-ge-imm" and w.wait_reg is None:
                    vc = wait_vc(prodvc, w)
                    if vc is not None:
                        for k, v in vc.items():
                            if myvc.get(k, 0) < v:
                                myvc[k] = v
                    if myvc.get(w.id, 0) < w.wait_value:
                        myvc[w.id] = w.wait_value
            procvc[proc] = myvc
            for u in ups:
                if u.update_mode in INC and u.update_reg is None:
                    cum[u.id] = cum.get(u.id, 0) + u.update_value
                    updaters.setdefault(u.id, set()).add(proc)
                else:
                    cum.pop(u.id, None)
                    updaters[u.id] = {object()}


def _legalize_waits(nc, keep=1, drain_cap=1):
    for fn in nc.m.functions:
        for bb in fn.blocks:
            newl = []
            for inst in bb.instructions:
                si = inst.sync_info
                waits = list(si.on_wait) if si is not None and si.on_wait else []
                if type(inst).__name__ in _WAIT_LIMITED and len(waits) > keep:
                    extra, kept = waits[:-keep], waits[-keep:]
                    for i in range(0, len(extra), drain_cap):
                        d = mybir.InstDrain(name=f"{inst.name}-wsplit{i}")
                        d.engine = inst.engine
                        d.sync_info = mybir.SyncInfo(
                            on_wait=extra[i : i + drain_cap], on_update=[])
                        newl.append(d)
                    inst.sync_info = mybir.SyncInfo(
                        on_wait=kept,
                        on_update=list(si.on_update) if si.on_update else [])
                newl.append(inst)
            bb.instructions[:] = newl
    return


_NC_CACHE = None


def build_nc(legalize=True):
    global _NC_CACHE
    if _NC_CACHE is not None:
        return _NC_CACHE
    nc = bass.Bass("TRN2", target_bir_lowering=False, debug=False)
    xt = nc.dram_tensor("xt", [NT, TOK_TILE // P, P, NG, P], f16, kind="ExternalInput").ap()
    w16 = nc.dram_tensor("w16", [P, NG, NW16], f16, kind="ExternalInput").ap()
    wf8 = nc.dram_tensor("wf8", [P, NPAIR, 2, NW8], f8, kind="ExternalInput").ap()
    biasv = nc.dram_tensor("biasv", [NSH], f32, kind="ExternalInput").ap()
    out = nc.dram_tensor("out", [M, NSH], f16, kind="ExternalOutput").ap()
    with tile.TileContext(nc) as tc:
        with ExitStack() as ctx:
            _kernel_body(ctx, tc, out, xt, w16, wf8, biasv)
    if legalize:
        _dedupe_ldweights(nc)
        _reduce_waits(nc)
        _legalize_waits(nc)
        _NC_CACHE = nc
    return nc


def prep_in_maps(inputs):
    """Host-side shard/layout prep.  Returns (in_maps, perm)."""
    x = np.asarray(inputs["x"], np.float32)
    hw = np.asarray(inputs["high_prec_weight"])
    hs = np.asarray(inputs["high_prec_scales"], np.float32)
    lw = np.asarray(inputs["low_prec_weight"])
    ls1 = np.asarray(inputs["low_prec_scales"], np.float32)
    ls2 = np.asarray(inputs["low_prec_scales2"], np.float32)
    lz = np.asarray(inputs["low_prec_zeros"], np.float32)
    perm = np.asarray(inputs["col_indices_inv"]).astype(np.int64)
    bias = np.asarray(inputs["bias"], np.float32)

    x2 = x.reshape(M, K)
    # x^T tiled per 128-token subtile:
    #   xt[t, s, p, g, m] = x[t*512 + s*128 + m, g*128 + p].
    xr = x2.reshape(NT, TOK_TILE // P, P, NG, P)
    xt = np.ascontiguousarray(xr.astype(np.float16).transpose(0, 1, 4, 3, 2))

    # Dequantized weights in concat order.
    w_high = (hw.astype(np.float32).reshape(NG, GROUP, N_HIGH)
              * hs[:, None, :]).reshape(K, N_HIGH)
    w_low = (lw.astype(np.float32) - lz) * ls1 * ls2

    bias_cat = np.empty(OUT_F, np.float32)
    bias_cat[perm] = bias  # bias_cat[col_inv[j]] = bias[j]

    in_maps = []
    for c in range(NCORES):
        hsl = slice(c * NH, (c + 1) * NH)
        lsl0 = c * NL            # stray 8 low cols -> fp16 path
        lsl8 = slice(c * NL + 8, (c + 1) * NL)  # 1024 low cols -> fp8 path
        w16c = np.concatenate(
            [w_high[:, hsl], w_low[:, lsl0:lsl0 + 8]], axis=1).astype(np.float16)
        w8c = w_low[:, lsl8].astype(ml_dtypes.float8_e4m3)
        biasv = np.concatenate(
            [bias_cat[c * NH:(c + 1) * NH],
             bias_cat[N_HIGH + c * NL:N_HIGH + (c + 1) * NL]]).astype(np.float32)
        in_maps.append({
            "xt": xt,
            "w16": np.ascontiguousarray(
                w16c.reshape(NG, P, NW16).transpose(1, 0, 2)),
            "wf8": np.ascontiguousarray(
                w8c.reshape(NPAIR, 2, P, NW8).transpose(2, 0, 1, 3)),
            "biasv": np.ascontiguousarray(biasv),
        })
    return in_maps, perm


def gather_output(results, perm):
    cat = np.empty((M, OUT_F), np.float32)
    for c in range(NCORES):
        o = results[c]["out"]
        cat[:, c * NH:(c + 1) * NH] = o[:, :NH]
        cat[:, N_HIGH + c * NL:N_HIGH + (c + 1) * NL] = o[:, NH:]
    final = np.take(cat, perm, axis=1)
    return np.ascontiguousarray(final.reshape(B, S, OUT_F).astype(np.float32))


def run(inputs, **spmd_kwargs):
    """Run on hardware; returns (output, BassKernelResults)."""
    nc = build_nc()
    in_maps, perm = prep_in_maps(inputs)
    res = run_bass_kernel_spmd(nc, in_maps, list(range(NCORES)), **spmd_kwargs)
    return gather_output(res.results, perm), res


def kernel(**inputs) -> np.ndarray:
    out, _ = run(inputs)
    return out



# revision 35
# speedup vs baseline: 1.1576x; 1.1576x over previous
"""CPRLinearMultiPrecision kernel for 8 TRN2 NeuronCores — fp8 DoubleRow.

The reference absmax is dominated by the high-precision columns (out std
~55 vs ~6 for the low columns), so the 2e-2 relative-error budget leaves
the low columns ~5 absolute error headroom.  That pays for an fp8
matmul on the low 75% of columns:

  * low cols:  W8 = e4m3((q - z[k]) * s1[k] * s2[c]) prepared on host,
    x8 = e4m3(x).  PE runs perf_mode=DoubleRow: both operands
    [128, 2, N] fp8, contracting TWO 128-row K-groups per column-cycle
    (~1.13 cyc/col vs 1.0 for fp16 but half the matmuls).  Measured
    numpy error: max |err| ~2.0 « 6.5 budget (rel ~6e-3 total).
  * high cols: exact int8*scale structure needs >4 significant bits, and
    a 2-pass fp8 split only ties fp16, so they stay an fp16 matmul with
    host-dequantized W.

Every core gets an equal mix (column-parallel, rebalanced): 344 high
cols + 8 stray low cols as the fp16 path (352 = one PSUM chunk) + 1024
low cols as the fp8 path (2 x 512 chunks).  Per 128-token subtile the
PE runs 32 fp16 matmuls (N=352, 149ns) + 16 DoubleRow pairs (2 matmuls
of N=512, 216ns — no measured DR streaming penalty), ~11.9us; subtiles
run in pairs (fp16 A, fp16 B, DR A, DR B — six PSUM banks hold exactly
one pair) to halve the fp16<->DR phase transitions; the fp8 x tile is
cast on-device by the DVE rather than DMA'd, and the DVE also drains
psum+bias -> fp16 output tiles.

t=0 is supply-bound: all of W (7.1MB) plus 4MB of x must land at the
~345GB/s aggregate HBM cap before the PE has anything to chew on, and
the HAM clock state machine makes this expensive twice over — the PE
boots at K=4/8 (half clock), upshifts only after ~3.2us of GAPLESS
matmul activity, and downshifts again on any ~2us idle.  Countermeasures
(worth ~7us at 2.4GHz vs the plain EDF prologue):
  * 32 dependency-free garbage matmuls right after the preamble warm the
    array to K=8/8 by ~13us and bridge the PE to the first real supply
    (the rings only start flowing at ~9.1/10.6/12.3us and share the cap
    roughly 85/105/160 GB/s once all active);
  * the first pair runs pair-chunk-major (A then B per 4-group w16
    chunk, x in NG-quarters; per-chunk stalls padded with 1-2 garbage
    matmuls stay under the downshift threshold), then chunk-major DR;
  * the second pair (C,D) is m-tile-major since x03 lands ~8us after
    x02; steady-state pairs then run exactly at the 23.36us streaming
    floor (measured 23.38).
The final subtile's last 512-col chain is evicted in 4x128-col slices
across both HWDGE rings so only ~1.3us of add+store rides the tail.

Host side: dequantize/quantize W, fold s1/s2/z into the fp8 codes (bias
is applied on device); gather = concat + column permutation.

Measured on 8 TRN2 cores: ~409us HW exec at 2.4GHz (vs 411.3us for the
plain-EDF baseline; PE streaming floor 373.8us + ~8us preamble+ramp +
~5us counted tail), rel err ~5.5e-3 vs the fp32 reference (budget
2e-2).  Beware run-to-run P0 downclock (PE upshifts to 2.0GHz instead
of 2.4): check MATMUL start-to-start is 149/216ns before comparing
timings.

Post-scheduling passes (unchanged from the fp16 baseline):
_dedupe_ldweights drops back-to-back LDWEIGHTS reloading an unchanged
stationary tile (the compiler-side --enable-ldw-opt is disabled);
_reduce_waits prunes transitively-implied semaphore waits; and
_legalize_waits moves surplus waits onto same-engine Drains (engine ISA
structs encode a single sync-wait slot).
"""

import numpy as np
import ml_dtypes
from contextlib import ExitStack

import concourse.bass as bass
import concourse.tile as tile
from concourse import mybir
from concourse.bass_utils import run_bass_kernel_spmd

# Problem shapes (hardcoded; kernel.py must be self-contained).
B, S = 2, 2048
M = B * S              # 4096 tokens
K = 4096               # in features
OUT_F = 11008
N_HIGH = 2752
N_LOW = OUT_F - N_HIGH  # 8256
GROUP = 128
NG = K // GROUP        # 32 K-groups
NPAIR = NG // 2        # 16 DoubleRow group-pairs
NCORES = 8
NH = N_HIGH // NCORES  # 344 high cols per core
NL = N_LOW // NCORES   # 1032 low cols per core
NW16 = NH + 8          # fp16-path cols per core (344 high + 8 stray low)
NW8 = NL - 8           # fp8-path cols per core (1024)
NSH = NW16 + NW8       # 1376 output cols per core
P = 128
TOK_TILE = 512
NT = M // TOK_TILE     # 8
GCHUNK = 2             # fp16 W groups per DMA chunk
PCHUNK = 2             # fp8 W pairs per DMA chunk

f8 = mybir.dt.float8e4
f16 = mybir.dt.float16
f32 = mybir.dt.float32

DR = mybir.MatmulPerfMode.DoubleRow


def _bcast(ap_1d, parts):
    """Partition-broadcast AP: replicate a 1-D DRAM row across `parts` partitions."""
    return bass.AP(tensor=ap_1d.tensor, offset=ap_1d.offset, ap=[[0, parts]] + ap_1d.ap)


def _kernel_body(ctx, tc, out, xt, w16, wf8, biasv):
    nc = tc.nc
    const = ctx.enter_context(tc.tile_pool(name="const", bufs=1))
    w16pool = ctx.enter_context(tc.tile_pool(name="w16", bufs=NG // GCHUNK))
    w8pool = ctx.enter_context(tc.tile_pool(name="w8", bufs=NPAIR // PCHUNK))
    xpool = ctx.enter_context(tc.tile_pool(name="x", bufs=8))
    xqpool = ctx.enter_context(tc.tile_pool(name="xq", bufs=8))
    x8pool = ctx.enter_context(tc.tile_pool(name="x8", bufs=8))
    opool = ctx.enter_context(tc.tile_pool(name="o", bufs=4))
    ppool = ctx.enter_context(tc.tile_pool(name="p", bufs=6, space="PSUM"))

    # t=0 needs ALL of W (7.1MB) plus its tt=0 x slices within its first
    # ~12us of PE work, and each DMA ring sustains only ~125GB/s draining
    # FIFO — so x is tiled per 128-token subtile (1.5MB/subtile instead of
    # 6.3MB/t upfront) and the t0-critical transfers are spread across all
    # three rings (sync/scalar HWDGE + gpsimd SWDGE) in PE consumption
    # order (earliest-deadline-first).
    w16t = [w16pool.tile([P, GCHUNK, NW16], f16, tag="w16", name=f"w16c{i}")
            for i in range(NG // GCHUNK)]
    w8t = [w8pool.tile([P, PCHUNK, 2, NW8], f8, tag="w8", name=f"w8c{i}")
           for i in range(NPAIR // PCHUNK)]
    garb = const.tile([P, 512], f16)
    bias_b = const.tile([P, NSH], f32)

    def load_x_tt(t, tt, eng=None):
        xc = xpool.tile([P, NG, P], f16, tag="xc")
        (eng or nc.sync).dma_start(out=xc[:], in_=xt[t, tt, :, :, :])
        return xc

    def cast_x8(xc):
        # fp8 copy of the x subtile for the DoubleRow path — derived
        # on-device (DVE) instead of a second HBM stream, keeping the
        # ring-bound t0 prologue lean.
        x8c = x8pool.tile([P, NG, P], f8, tag="x8c")
        nc.vector.tensor_copy(x8c[:], xc[:])
        return x8c

    # --- PE clock warm-up -------------------------------------------------
    # The HAM state machine starts the PE at K=4/8 (half clock) and only
    # upshifts after ~3.2us of GAPLESS matmul activity; any >=0.5us idle
    # resets the accumulator, and t0 is full of supply stalls — on the
    # baseline the array stayed at half clock until t~42us.  A burst of
    # dependency-free garbage matmuls right after the framework preamble
    # (PE idle until ~18us otherwise: DMA rings only start flowing at
    # ~9.6us) upshifts the clock by ~12us, before the first real matmul.
    nc.vector.memset(garb[:], 1.0)
    dummy_ps = ppool.tile([P, 512], f32, tag="ps", name="warm")

    def dummies(n):
        for _ in range(n):
            nc.tensor.matmul(dummy_ps[:, :], garb[:, :P], garb[:, :],
                             start=True, stop=True)

    dummies(32)

    # Prologue, spread EDF-style over the three rings in PE consumption
    # order.  The fp16 phase of the first pair runs pair-chunk-major, so
    # its critical supply (w16c0 + x(0,0)/x(0,1) in NG-quarters) leads the
    # fast-starting sync ring; w16c1..7 ride scalar; wf8 is split so the
    # chunk-major DR(A,B) phase never waits:
    #   sync:   w16c0, x00/x01 quarters (interleaved), wf8c5-6, x02, x03
    #   scalar: w16c1..4, bias, w16c5..7
    #   gpsimd: wf8c0..4, wf8c7
    def load_w8(i, eng):
        eng.dma_start(out=w8t[i][:], in_=wf8[:, i * PCHUNK:(i + 1) * PCHUNK, :, :])

    # EDF split across the three rings using their measured shares when all
    # are active (sync ~85GB/s, scalar ~105, gpsimd-SWDGE ~160; the SWDGE
    # ring starts ~3.5us later but drains fastest).  Deadline order is the
    # PE's: w16c+x-quarters paced ~1.4us/chunk from ~15us, then wf8 chunks
    # ~2us apart from ~29us, then x02/x03/bias for the C,D pair.
    def load_w16(i, eng):
        eng.dma_start(out=w16t[i][:], in_=w16[:, i * GCHUNK:(i + 1) * GCHUNK, :])

    def load_xq(tt, j, eng):
        q = xqpool.tile([P, 8, P], f16, tag="xq")
        eng.dma_start(out=q[:], in_=xt[0, tt, :, 8 * j:8 * j + 8, :])
        return q

    xq = [[None] * 4 for _ in range(2)]  # xq[tt][j]: groups 8j..8j+7 of x(0,tt)
    # sync: first-matmul critical path (w16c0 + all x quarters), then the
    # late wf8 chunks and x03
    load_w16(0, nc.sync)
    load_w16(1, nc.sync)
    for j in range(4):
        xq[0][j] = load_xq(0, j, nc.sync)
        xq[1][j] = load_xq(1, j, nc.sync)
    for i in (5, 6):
        load_w8(i, nc.sync)
    x02 = load_x_tt(0, 2, nc.sync)
    x03 = load_x_tt(0, 3, nc.sync)
    # scalar: the rest of w16, bias mid-stream
    for i in (2, 3, 4, 5, 6, 7, 8, 9):
        load_w16(i, nc.scalar)
    nc.scalar.dma_start(out=bias_b[:], in_=_bcast(biasv[:], P))
    for i in (10, 11, 12, 13, 14, 15):
        load_w16(i, nc.scalar)
    # gpsimd (fast once its ~3.5us SWDGE spin-up passes): the early wf8
    # chunks (consumed from ~28us).  NOTE: shifting MORE onto the SWDGE
    # ring (all 8 wf8 chunks, or w16/x tiles) measured consistently worse
    # — its effective rate degrades with queue depth, and the HWDGE/SWDGE
    # share split drifts ±30% run-to-run, so this EDF split was tuned
    # empirically, not from a rate model.
    for i in (0, 1, 2, 3, 4, 7):
        load_w8(i, nc.gpsimd)
    cur23 = [x02, x03]

    def fp16_chain(xc, ps0):
        # stationary x [128k x 128tok], moving W16 [128k x 352]
        for g in range(NG):
            nc.tensor.matmul(
                ps0[:, :], xc[:, g, :],
                w16t[g // GCHUNK][:, g % GCHUNK, :],
                start=(g == 0), stop=(g == NG - 1),
            )

    def dr_chains(x8c, ps1, ps2, order):
        # fp8 DoubleRow: both operands [128, 2, free]; contracts groups
        # (2p, 2p+1) per column-cycle.
        for p, ci in order:
            c0, pst = ((0, ps1), (512, ps2))[ci]
            nc.tensor.matmul(
                pst[:, :], x8c[:, 2 * p:2 * p + 2, :],
                w8t[p // PCHUNK][:, p % PCHUNK, :, c0:c0 + 512],
                start=(p == 0), stop=(p == NPAIR - 1),
                perf_mode=DR,
            )

    def evict(t, tt, ps, last):
        row0 = t * TOK_TILE + tt * P
        osb = opool.tile([P, NSH], f16, tag="osb")
        if not last:
            for c0, cw, pst in ((0, NW16, ps[0]), (NW16, 512, ps[1]),
                                (NW16 + 512, 512, ps[2])):
                nc.vector.tensor_add(osb[:, c0:c0 + cw], pst[:],
                                     bias_b[:, c0:c0 + cw])
            nc.gpsimd.dma_start(out=out[row0:row0 + P, :], in_=osb[:])
            return
        # Final subtile: first two chunks drain while the sliced last DR
        # chain still streams; the 512-col tail goes out in 4x128 slices
        # alternating across both HWDGE rings so only ~1us of add+store
        # trails the last matmul.
        for c0, cw, pst in ((0, NW16, ps[0]), (NW16, 512, ps[1])):
            nc.vector.tensor_add(osb[:, c0:c0 + cw], pst[:], bias_b[:, c0:c0 + cw])
            nc.scalar.dma_start(out=out[row0:row0 + P, c0:c0 + cw],
                                in_=osb[:, c0:c0 + cw])
        for s in range(4):
            c0 = NW16 + 512 + 128 * s
            nc.vector.tensor_add(osb[:, c0:c0 + 128],
                                 ps[2][:, 128 * s:128 * s + 128],
                                 bias_b[:, c0:c0 + 128])
            eng = (nc.sync, nc.scalar)[s % 2]
            eng.dma_start(out=out[row0:row0 + P, c0:c0 + 128],
                          in_=osb[:, c0:c0 + 128])

    def alloc_ps():
        return [ppool.tile([P, NW16], f32, tag="ps", name="ps0"),
                ppool.tile([P, 512], f32, tag="ps", name="ps1"),
                ppool.tile([P, 512], f32, tag="ps", name="ps2")]

    # --- t0 pair (A,B): supply-paced chunk-major ------------------------
    # The first pair is DMA-supply-bound (all of W plus 4MB of x must
    # stream in at ~370GB/s aggregate while the PE wants to run).  Both
    # fp16 chains run pair-chunk-major (A then B per 4-group w16 chunk, x
    # in NG-quarters) and both DR chains pair-chunk-major per wf8 chunk,
    # so each arriving chunk unlocks ~1.2-1.7us of work and no single
    # stall exceeds the ~3us HAM downshift threshold.
    def cast_x8_q(qs):
        x8c = x8pool.tile([P, NG, P], f8, tag="x8c")
        for j in range(4):
            nc.vector.tensor_copy(x8c[:, 8 * j:8 * j + 8, :], qs[j][:])
        return x8c

    x8A, x8B = cast_x8_q(xq[0]), cast_x8_q(xq[1])
    psA, psB = alloc_ps(), alloc_ps()
    for c in range(NG // GCHUNK):
        for tt in (0, 1):
            ps0 = (psA, psB)[tt][0]
            for g4 in range(GCHUNK):
                g = GCHUNK * c + g4
                j = g // 8
                nc.tensor.matmul(
                    ps0[:, :], xq[tt][j][:, g - 8 * j, :],
                    w16t[c][:, g4, :],
                    start=(g == 0), stop=(g == NG - 1),
                )
        if c % 2 == 1 and c < NG // GCHUNK - 1:
            # keep the supply-paced gap under the ~1.9us HAM downshift
            # threshold while the next w16 chunks stream in (pad per
            # 360KB of scalar-ring supply, as at GCHUNK=4)
            dummies(1 if c < 7 else 2)


    for pc in range(NPAIR // PCHUNK):
        for tt in (0, 1):
            x8c = (x8A, x8B)[tt]
            ps1, ps2 = (psA, psB)[tt][1], (psA, psB)[tt][2]
            for pp in range(PCHUNK * pc, PCHUNK * (pc + 1)):
                for c0, pst in ((0, ps1), (512, ps2)):
                    nc.tensor.matmul(
                        pst[:, :], x8c[:, 2 * pp:2 * pp + 2, :],
                        w8t[pc][:, pp % PCHUNK, :, c0:c0 + 512],
                        start=(pp == 0), stop=(pp == NPAIR - 1),
                        perf_mode=DR,
                    )
    evict(0, 0, psA, False)
    evict(0, 1, psB, False)

    # --- t0 pair (C,D): m-tile-major ------------------------------------
    # x02/x03 land late (~38/46us) on the saturated rings, so C runs to
    # completion (its x tile arrives first) before D touches x03; the two
    # extra fp16<->DR transitions cost ~64ns each.
    x8C = cast_x8(cur23[0])
    psC = alloc_ps()
    fp16_chain(cur23[0], psC[0])
    nxt = [load_x_tt(1, s) for s in range(TOK_TILE // P)]
    dr_chains(x8C, psC[1], psC[2],
              [(p, ci) for p in range(NPAIR) for ci in (0, 1)])
    evict(0, 2, psC, False)
    x8D = cast_x8(cur23[1])
    psD = alloc_ps()
    fp16_chain(cur23[1], psD[0])
    dr_chains(x8D, psD[1], psD[2],
              [(p, ci) for p in range(NPAIR) for ci in (0, 1)])
    evict(0, 3, psD, False)

    # --- t=1..7: steady-state pairs -------------------------------------
    # Subtiles run in PAIRS — fp16(A), fp16(B), DR(A), DR(B).  Six PSUM
    # banks hold exactly one pair's three chains x2.
    cur = nxt
    for t in range(1, NT):
        for pr in range(2):
            ttA, ttB = 2 * pr, 2 * pr + 1
            # Prefetch next t's x tiles during the first pair.
            if pr == 0 and t + 1 < NT:
                nxt = [load_x_tt(t + 1, s) for s in range(TOK_TILE // P)]
            x8A, x8B = cast_x8(cur[ttA]), cast_x8(cur[ttB])
            psA, psB = alloc_ps(), alloc_ps()
            last = (t == NT - 1) and (pr == 1)
            fp16_chain(cur[ttA], psA[0])
            fp16_chain(cur[ttB], psB[0])
            dr_chains(x8A, psA[1], psA[2],
                      [(p, ci) for p in range(NPAIR) for ci in (0, 1)])
            evict(t, ttA, psA, False)
            if not last:
                dr_chains(x8B, psB[1], psB[2],
                          [(p, ci) for p in range(NPAIR) for ci in (0, 1)])
            else:
                # Final subtile: sequential DR chains so ps1 drains through
                # DVE/DMA while ps2 still streams — shortens the tail.
                # (Column-slicing the ps2 CHAIN is unsound: interleaved
                # accumulation chains in one PSUM bank clobber each other's
                # partials via the start_tensor_calc bank clear; only the
                # EVICTION is sliced, in evict().)
                dr_chains(x8B, psB[1], psB[2], [(p, 0) for p in range(NPAIR)])
                dr_chains(x8B, psB[1], psB[2], [(p, 1) for p in range(NPAIR)])
            evict(t, ttB, psB, last)
        cur = nxt if t + 1 < NT else None


# Engine-compute ISA structs encode very few sync-wait slots (the DVE
# tensor ops hold only one); walrus codegen hard-fails on excess.  Tile's
# scheduler may attach several waits to one instruction, so after
# scheduling we move the surplus onto same-engine Drain instructions
# inserted immediately before (the engine stalls there instead — same
# semantics, and drains legally carry many waits).
_WAIT_LIMITED = {
    "InstTensorTensor", "InstTensorScalarPtr", "InstTensorCopy",
    "InstTensorReduce", "InstMemset", "InstActivation", "InstIota",
    "InstMatmult", "InstLdweights", "InstBNStats", "InstBNStatsAggregate",
    "InstDrain", "InstDMACopy",
}


def _dedupe_ldweights(nc):
    """Delete back-to-back redundant LDWEIGHTS.

    The two column-chunk matmuls of each DoubleRow pair share one
    stationary tile, but bass emits an Ldweights per matmul and the
    compiler-side dedup (--enable-ldw-opt) is disabled.  Reloading
    identical weights is idempotent, so an Ldweights whose source AP
    equals the previous one on the PE stream — with only matmuls in
    between, no sync waits and no sem updates of its own — can be
    dropped.  W tiles are written once and never recycled, and x-tile
    slot reuse is gated on the matmuls' sem increments (Ldweights never
    increments), so sem bookkeeping is unchanged.
    """
    removed = 0
    for fn in nc.m.functions:
        for bb in fn.blocks:
            newl = []
            prev_ldw_key = None
            for inst in bb.instructions:
                t = type(inst).__name__
                eng = str(inst.engine)
                if eng == "EngineType.PE":
                    if t == "InstLdweights":
                        si = inst.sync_info
                        has_sync = si is not None and (si.on_wait or si.on_update)
                        key = str(inst.ins)
                        if key == prev_ldw_key and not has_sync:
                            removed += 1
                            continue
                        prev_ldw_key = key
                    elif t != "InstMatmult":
                        prev_ldw_key = None
                newl.append(inst)
            bb.instructions[:] = newl
    return removed


def _reduce_waits(nc):
    """Drop transitively-implied semaphore waits.

    A wait (sem, v) on instruction X is redundant when another wait on X
    targets a producer whose vector clock already covers (sem, v), when
    X's own proc has already observed it, or when the sem belongs to X's
    own in-order proc (same-FIFO dominance).  Two phases: build complete
    per-sem producer vector clocks (block list order is per-proc
    consistent; cross-proc misses only make the result conservative),
    then prune using the final maps.  Only 'sem-ge-imm' waits and
    incrementing ('sem-inc'/'sem-add-imm') updates participate; any other
    update mode invalidates that sem's history.
    """
    INC = ("sem-inc", "sem-add-imm")
    for fn in nc.m.functions:
        insts = [inst for bb in fn.blocks for inst in bb.instructions]

        def params(inst):
            si = inst.sync_info
            waits = list(si.on_wait) if si is not None and si.on_wait else []
            ups = list(si.on_update) if si is not None and si.on_update else []
            proc = (str(inst.engine), getattr(inst, "bass_scheduled_proc", None))
            return si, waits, ups, proc

        def wait_vc(prodvc, w):
            if w.wait_mode != "sem-ge-imm" or w.wait_reg is not None:
                return None
            for cv, vc in prodvc.get(w.id, []):
                if cv >= w.wait_value:
                    return vc
            return None

        def build_maps(lookup_prodvc):
            cum, prodvc, procvc, updaters, obsvc = {}, {}, {}, {}, {}
            for inst in insts:
                si, waits, ups, proc = params(inst)
                myvc = dict(procvc.get(proc, {}))
                for w in waits:
                    if w.wait_mode == "sem-ge-imm" and w.wait_reg is None:
                        vc = wait_vc(lookup_prodvc if lookup_prodvc is not None
                                     else prodvc, w)
                        if vc is not None:
                            for k, v in vc.items():
                                if myvc.get(k, 0) < v:
                                    myvc[k] = v
                        if myvc.get(w.id, 0) < w.wait_value:
                            myvc[w.id] = w.wait_value
                procvc[proc] = myvc
                obsvc[id(inst)] = myvc
                for u in ups:
                    if u.update_mode in INC and u.update_reg is None:
                        cum[u.id] = cum.get(u.id, 0) + u.update_value
                        updaters.setdefault(u.id, set()).add(proc)
                        snap = dict(myvc)
                        snap[u.id] = cum[u.id]
                        prodvc.setdefault(u.id, []).append((cum[u.id], snap))
                    else:
                        cum.pop(u.id, None)
                        prodvc.pop(u.id, None)
                        updaters[u.id] = {object()}
            return cum, prodvc, updaters, obsvc

        # Pass 1 builds conservative clocks; pass 2 rebuilds them resolving
        # forward references through pass 1's complete producer map.
        _, prodvc, _, _ = build_maps(None)
        _, prodvc, _, _ = build_maps(prodvc)

        # Prune with the final maps, tracking per-proc observation and
        # per-proc cumulative sem updates in list order.
        cum, procvc, updaters = {}, {}, {}
        for inst in insts:
            si, waits, ups, proc = params(inst)
            myvc = dict(procvc.get(proc, {}))
            if len(waits) > 1:
                vcs = [wait_vc(prodvc, w) for w in waits]
                keep_ws = []
                for i, w in enumerate(waits):
                    if w.wait_mode == "sem-ge-imm" and w.wait_reg is None:
                        if myvc.get(w.id, 0) >= w.wait_value:
                            continue
                        if (updaters.get(w.id) == {proc}
                                and cum.get(w.id, 0) >= w.wait_value):
                            continue
                        implied = any(
                            j != i and vcs[j] is not None
                            and vcs[j].get(w.id, 0) >= w.wait_value
                            for j in range(len(waits)))
                        if implied:
                            continue
                    keep_ws.append(w)
                if len(keep_ws) != len(waits):
                    inst.sync_info = mybir.SyncInfo(on_wait=keep_ws, on_update=ups)
                    waits = keep_ws
            for w in waits:
                if w.wait_mode == "sem-ge-imm" and w.wait_reg is None:
                    vc = wait_vc(prodvc, w)
                    if vc is not None:
                        for k, v in vc.items():
                            if myvc.get(k, 0) < v:
                                myvc[k] = v
                    if myvc.get(w.id, 0) < w.wait_value:
                        myvc[w.id] = w.wait_value
            procvc[proc] = myvc
            for u in ups:
                if u.update_mode in INC and u.update_reg is None:
                    cum[u.id] = cum.get(u.id, 0) + u.update_value
                    updaters.setdefault(u.id, set()).add(proc)
                else:
                    cum.pop(u.id, None)
                    updaters[u.id] = {object()}


def _legalize_waits(nc, keep=1, drain_cap=1):
    for fn in nc.m.functions:
        for bb in fn.blocks:
            newl = []
            for inst in bb.instructions:
                si = inst.sync_info
                waits = list(si.on_wait) if si is not None and si.on_wait else []
                if type(inst).__name__ in _WAIT_LIMITED and len(waits) > keep:
                    extra, kept = waits[:-keep], waits[-keep:]
                    for i in range(0, len(extra), drain_cap):
                        d = mybir.InstDrain(name=f"{inst.name}-wsplit{i}")
                        d.engine = inst.engine
                        d.sync_info = mybir.SyncInfo(
                            on_wait=extra[i : i + drain_cap], on_update=[])
                        newl.append(d)
                    inst.sync_info = mybir.SyncInfo(
                        on_wait=kept,
                        on_update=list(si.on_update) if si.on_update else [])
                newl.append(inst)
            bb.instructions[:] = newl
    return


_NC_CACHE = None


def build_nc(legalize=True):
    global _NC_CACHE
    if _NC_CACHE is not None:
        return _NC_CACHE
    nc = bass.Bass("TRN2", target_bir_lowering=False, debug=False)
    xt = nc.dram_tensor("xt", [NT, TOK_TILE // P, P, NG, P], f16, kind="ExternalInput").ap()
    w16 = nc.dram_tensor("w16", [P, NG, NW16], f16, kind="ExternalInput").ap()
    wf8 = nc.dram_tensor("wf8", [P, NPAIR, 2, NW8], f8, kind="ExternalInput").ap()
    biasv = nc.dram_tensor("biasv", [NSH], f32, kind="ExternalInput").ap()
    out = nc.dram_tensor("out", [M, NSH], f16, kind="ExternalOutput").ap()
    with tile.TileContext(nc) as tc:
        with ExitStack() as ctx:
            _kernel_body(ctx, tc, out, xt, w16, wf8, biasv)
    if legalize:
        _dedupe_ldweights(nc)
        _reduce_waits(nc)
        _legalize_waits(nc)
        _NC_CACHE = nc
    return nc


def prep_in_maps(inputs):
    """Host-side shard/layout prep.  Returns (in_maps, perm)."""
    x = np.asarray(inputs["x"], np.float32)
    hw = np.asarray(inputs["high_prec_weight"])
    hs = np.asarray(inputs["high_prec_scales"], np.float32)
    lw = np.asarray(inputs["low_prec_weight"])
    ls1 = np.asarray(inputs["low_prec_scales"], np.float32)
    ls2 = np.asarray(inputs["low_prec_scales2"], np.float32)
    lz = np.asarray(inputs["low_prec_zeros"], np.float32)
    perm = np.asarray(inputs["col_indices_inv"]).astype(np.int64)
    bias = np.asarray(inputs["bias"], np.float32)

    x2 = x.reshape(M, K)
    # x^T tiled per 128-token subtile:
    #   xt[t, s, p, g, m] = x[t*512 + s*128 + m, g*128 + p].
    xr = x2.reshape(NT, TOK_TILE // P, P, NG, P)
    xt = np.ascontiguousarray(xr.astype(np.float16).transpose(0, 1, 4, 3, 2))

    # Dequantized weights in concat order.
    w_high = (hw.astype(np.float32).reshape(NG, GROUP, N_HIGH)
              * hs[:, None, :]).reshape(K, N_HIGH)
    w_low = (lw.astype(np.float32) - lz) * ls1 * ls2

    bias_cat = np.empty(OUT_F, np.float32)
    bias_cat[perm] = bias  # bias_cat[col_inv[j]] = bias[j]

    in_maps = []
    for c in range(NCORES):
        hsl = slice(c * NH, (c + 1) * NH)
        lsl0 = c * NL            # stray 8 low cols -> fp16 path
        lsl8 = slice(c * NL + 8, (c + 1) * NL)  # 1024 low cols -> fp8 path
        w16c = np.concatenate(
            [w_high[:, hsl], w_low[:, lsl0:lsl0 + 8]], axis=1).astype(np.float16)
        w8c = w_low[:, lsl8].astype(ml_dtypes.float8_e4m3)
        biasv = np.concatenate(
            [bias_cat[c * NH:(c + 1) * NH],
             bias_cat[N_HIGH + c * NL:N_HIGH + (c + 1) * NL]]).astype(np.float32)
        in_maps.append({
            "xt": xt,
            "w16": np.ascontiguousarray(
                w16c.reshape(NG, P, NW16).transpose(1, 0, 2)),
            "wf8": np.ascontiguousarray(
                w8c.reshape(NPAIR, 2, P, NW8).transpose(2, 0, 1, 3)),
            "biasv": np.ascontiguousarray(biasv),
        })
    return in_maps, perm


def gather_output(results, perm):
    cat = np.empty((M, OUT_F), np.float32)
    for c in range(NCORES):
        o = results[c]["out"]
        cat[:, c * NH:(c + 1) * NH] = o[:, :NH]
        cat[:, N_HIGH + c * NL:N_HIGH + (c + 1) * NL] = o[:, NH:]
    final = np.take(cat, perm, axis=1)
    return np.ascontiguousarray(final.reshape(B, S, OUT_F).astype(np.float32))


def run(inputs, **spmd_kwargs):
    """Run on hardware; returns (output, BassKernelResults)."""
    nc = build_nc()
    in_maps, perm = prep_in_maps(inputs)
    res = run_bass_kernel_spmd(nc, in_maps, list(range(NCORES)), **spmd_kwargs)
    return gather_output(res.results, perm), res


def kernel(**inputs) -> np.ndarray:
    out, _ = run(inputs)
    return out



# revision 36
# speedup vs baseline: 1.1890x; 1.0272x over previous
"""CPRLinearMultiPrecision kernel for 8 TRN2 NeuronCores — fp8 DoubleRow.

The reference absmax is dominated by the high-precision columns (out std
~55 vs ~6 for the low columns), so the 2e-2 relative-error budget leaves
the low columns ~5 absolute error headroom.  That pays for an fp8
matmul on the low 75% of columns:

  * low cols:  W8 = e4m3((q - z[k]) * s1[k] * s2[c]) prepared on host,
    x8 = e4m3(x).  PE runs perf_mode=DoubleRow: both operands
    [128, 2, N] fp8, contracting TWO 128-row K-groups per column-cycle
    (~1.13 cyc/col vs 1.0 for fp16 but half the matmuls).  Measured
    numpy error: max |err| ~2.0 « 6.5 budget (rel ~6e-3 total).
  * high cols: exact int8*scale structure needs >4 significant bits, and
    a 2-pass fp8 split only ties fp16, so they stay an fp16 matmul with
    host-dequantized W.

Every core gets an equal mix (column-parallel, rebalanced): 344 high
cols + 8 stray low cols as the fp16 path (352 = one PSUM chunk) + 1024
low cols as the fp8 path (2 x 512 chunks).  Per 128-token subtile the
PE runs 32 fp16 matmuls (N=352, 149ns) + 16 DoubleRow pairs (2 matmuls
of N=512, 216ns — no measured DR streaming penalty), ~11.9us; subtiles
run in pairs (fp16 A, fp16 B, DR A, DR B — six PSUM banks hold exactly
one pair) to halve the fp16<->DR phase transitions; the fp8 x tile is
cast on-device by the DVE rather than DMA'd, and the DVE also drains
psum+bias -> fp16 output tiles.

t=0 is supply-bound: all of W (7.1MB) plus 4MB of x must land at the
~345GB/s aggregate HBM cap before the PE has anything to chew on, and
the HAM clock state machine makes this expensive twice over — the PE
boots at K=4/8 (half clock), upshifts only after ~3.2us of GAPLESS
matmul activity, and downshifts again on any ~2us idle.  Countermeasures
(worth ~7us at 2.4GHz vs the plain EDF prologue):
  * 32 dependency-free garbage matmuls right after the preamble warm the
    array to K=8/8 by ~13us and bridge the PE to the first real supply
    (the rings only start flowing at ~9.1/10.6/12.3us and share the cap
    roughly 85/105/160 GB/s once all active);
  * the first pair runs pair-chunk-major (A then B per 4-group w16
    chunk, x in NG-quarters; per-chunk stalls padded with 1-2 garbage
    matmuls stay under the downshift threshold), then chunk-major DR;
  * the second pair (C,D) is m-tile-major since x03 lands ~8us after
    x02; steady-state pairs then run exactly at the 23.36us streaming
    floor (measured 23.38).
The final subtile's last 512-col chain is evicted in 4x128-col slices
across both HWDGE rings so only ~1.3us of add+store rides the tail.

Host side: dequantize/quantize W, fold s1/s2/z into the fp8 codes (bias
is applied on device); gather = concat + column permutation.

Measured on 8 TRN2 cores: ~409us HW exec at 2.4GHz (vs 411.3us for the
plain-EDF baseline; PE streaming floor 373.8us + ~8us preamble+ramp +
~5us counted tail), rel err ~5.5e-3 vs the fp32 reference (budget
2e-2).  Beware run-to-run P0 downclock (PE upshifts to 2.0GHz instead
of 2.4): check MATMUL start-to-start is 149/216ns before comparing
timings.

Post-scheduling passes (unchanged from the fp16 baseline):
_dedupe_ldweights drops back-to-back LDWEIGHTS reloading an unchanged
stationary tile (the compiler-side --enable-ldw-opt is disabled);
_reduce_waits prunes transitively-implied semaphore waits; and
_legalize_waits moves surplus waits onto same-engine Drains (engine ISA
structs encode a single sync-wait slot).
"""

import numpy as np
import ml_dtypes
from contextlib import ExitStack

import concourse.bass as bass
import concourse.tile as tile
from concourse import mybir
from concourse.bass_utils import run_bass_kernel_spmd

# Problem shapes (hardcoded; kernel.py must be self-contained).
B, S = 2, 2048
M = B * S              # 4096 tokens
K = 4096               # in features
OUT_F = 11008
N_HIGH = 2752
N_LOW = OUT_F - N_HIGH  # 8256
GROUP = 128
NG = K // GROUP        # 32 K-groups
NPAIR = NG // 2        # 16 DoubleRow group-pairs
NCORES = 8
NH = N_HIGH // NCORES  # 344 high cols per core
NL = N_LOW // NCORES   # 1032 low cols per core
NW16 = NH + 8          # fp16-path cols per core (344 high + 8 stray low)
NW8 = NL - 8           # fp8-path cols per core (1024)
NSH = NW16 + NW8       # 1376 output cols per core
P = 128
TOK_TILE = 512
NT = M // TOK_TILE     # 8
GCHUNK = 4             # fp16 W groups per DMA chunk
PCHUNK = 2             # fp8 W pairs per DMA chunk

f8 = mybir.dt.float8e4
f16 = mybir.dt.float16
f32 = mybir.dt.float32

DR = mybir.MatmulPerfMode.DoubleRow


def _bcast(ap_1d, parts):
    """Partition-broadcast AP: replicate a 1-D DRAM row across `parts` partitions."""
    return bass.AP(tensor=ap_1d.tensor, offset=ap_1d.offset, ap=[[0, parts]] + ap_1d.ap)


def _kernel_body(ctx, tc, out, xt, w16, wf8, biasv):
    nc = tc.nc
    const = ctx.enter_context(tc.tile_pool(name="const", bufs=1))
    w16pool = ctx.enter_context(tc.tile_pool(name="w16", bufs=NG // GCHUNK))
    w8pool = ctx.enter_context(tc.tile_pool(name="w8", bufs=NPAIR // PCHUNK))
    xpool = ctx.enter_context(tc.tile_pool(name="x", bufs=8))
    xqpool = ctx.enter_context(tc.tile_pool(name="xq", bufs=8))
    x8pool = ctx.enter_context(tc.tile_pool(name="x8", bufs=8))
    opool = ctx.enter_context(tc.tile_pool(name="o", bufs=4))
    ppool = ctx.enter_context(tc.tile_pool(name="p", bufs=6, space="PSUM"))

    # t=0 needs ALL of W (7.1MB) plus its tt=0 x slices within its first
    # ~12us of PE work, and each DMA ring sustains only ~125GB/s draining
    # FIFO — so x is tiled per 128-token subtile (1.5MB/subtile instead of
    # 6.3MB/t upfront) and the t0-critical transfers are spread across all
    # three rings (sync/scalar HWDGE + gpsimd SWDGE) in PE consumption
    # order (earliest-deadline-first).
    w16t = [w16pool.tile([P, GCHUNK, NW16], f16, tag="w16", name=f"w16c{i}")
            for i in range(NG // GCHUNK)]
    w8t = [w8pool.tile([P, PCHUNK, 2, NW8], f8, tag="w8", name=f"w8c{i}")
           for i in range(NPAIR // PCHUNK)]
    garb = const.tile([P, 512], f16)
    bias_b = const.tile([P, NSH], f32)

    def load_x_tt(t, tt, eng=None):
        xc = xpool.tile([P, NG, P], f16, tag="xc")
        (eng or nc.sync).dma_start(out=xc[:], in_=xt[t, tt, :, :, :])
        return xc

    def cast_x8(xc):
        # fp8 copy of the x subtile for the DoubleRow path — derived
        # on-device (DVE) instead of a second HBM stream, keeping the
        # ring-bound t0 prologue lean.
        x8c = x8pool.tile([P, NG, P], f8, tag="x8c")
        nc.vector.tensor_copy(x8c[:], xc[:])
        return x8c

    # --- PE clock warm-up -------------------------------------------------
    # The HAM state machine starts the PE at K=4/8 (half clock) and only
    # upshifts after ~3.2us of GAPLESS matmul activity; any >=0.5us idle
    # resets the accumulator, and t0 is full of supply stalls — on the
    # baseline the array stayed at half clock until t~42us.  A burst of
    # dependency-free garbage matmuls right after the framework preamble
    # (PE idle until ~18us otherwise: DMA rings only start flowing at
    # ~9.6us) upshifts the clock by ~12us, before the first real matmul.
    nc.vector.memset(garb[:], 1.0)
    dummy_ps = ppool.tile([P, 512], f32, tag="ps", name="warm")

    def dummies(n):
        for _ in range(n):
            nc.tensor.matmul(dummy_ps[:, :], garb[:, :P], garb[:, :],
                             start=True, stop=True)

    dummies(32)

    # Prologue, spread EDF-style over the three rings in PE consumption
    # order.  The fp16 phase of the first pair runs pair-chunk-major, so
    # its critical supply (w16c0 + x(0,0)/x(0,1) in NG-quarters) leads the
    # fast-starting sync ring; w16c1..7 ride scalar; wf8 is split so the
    # chunk-major DR(A,B) phase never waits:
    #   sync:   w16c0, x00/x01 quarters (interleaved), wf8c5-6, x02, x03
    #   scalar: w16c1..4, bias, w16c5..7
    #   gpsimd: wf8c0..4, wf8c7
    def load_w8(i, eng):
        eng.dma_start(out=w8t[i][:], in_=wf8[:, i * PCHUNK:(i + 1) * PCHUNK, :, :])

    # EDF split across the three rings using their measured shares when all
    # are active (sync ~85GB/s, scalar ~105, gpsimd-SWDGE ~160; the SWDGE
    # ring starts ~3.5us later but drains fastest).  Deadline order is the
    # PE's: w16c+x-quarters paced ~1.4us/chunk from ~15us, then wf8 chunks
    # ~2us apart from ~29us, then x02/x03/bias for the C,D pair.
    def load_w16(i, eng):
        eng.dma_start(out=w16t[i][:], in_=w16[:, i * GCHUNK:(i + 1) * GCHUNK, :])

    def load_xq(tt, j, eng):
        q = xqpool.tile([P, 8, P], f16, tag="xq")
        eng.dma_start(out=q[:], in_=xt[0, tt, :, 8 * j:8 * j + 8, :])
        return q

    xq = [[None] * 4 for _ in range(2)]  # xq[tt][j]: groups 8j..8j+7 of x(0,tt)
    # sync: first-matmul critical path (w16c0 + all x quarters), then the
    # late wf8 chunks and x03
    load_w16(0, nc.sync)
    for j in range(4):
        xq[0][j] = load_xq(0, j, nc.sync)
        xq[1][j] = load_xq(1, j, nc.sync)
    for i in (5, 6):
        load_w8(i, nc.sync)
    x02 = load_x_tt(0, 2, nc.sync)
    x03 = load_x_tt(0, 3, nc.sync)
    # scalar: the rest of w16, bias mid-stream
    for i in (1, 2, 3, 4):
        load_w16(i, nc.scalar)
    nc.scalar.dma_start(out=bias_b[:], in_=_bcast(biasv[:], P))
    for i in (5, 6, 7):
        load_w16(i, nc.scalar)
    # gpsimd (fast once its ~3.5us SWDGE spin-up passes): the early wf8
    # chunks (consumed from ~28us).  NOTE: shifting MORE onto the SWDGE
    # ring (all 8 wf8 chunks, or w16/x tiles) measured consistently worse
    # — its effective rate degrades with queue depth, and the HWDGE/SWDGE
    # share split drifts ±30% run-to-run, so this EDF split was tuned
    # empirically, not from a rate model.
    for i in (0, 1, 2, 3, 4, 7):
        load_w8(i, nc.gpsimd)
    cur23 = [x02, x03]

    def fp16_chain(xc, ps0):
        # stationary x [128k x 128tok], moving W16 [128k x 352]
        for g in range(NG):
            nc.tensor.matmul(
                ps0[:, :], xc[:, g, :],
                w16t[g // GCHUNK][:, g % GCHUNK, :],
                start=(g == 0), stop=(g == NG - 1),
            )

    def dr_chains(x8c, ps1, ps2, order):
        # fp8 DoubleRow: both operands [128, 2, free]; contracts groups
        # (2p, 2p+1) per column-cycle.
        for p, ci in order:
            c0, pst = ((0, ps1), (512, ps2))[ci]
            nc.tensor.matmul(
                pst[:, :], x8c[:, 2 * p:2 * p + 2, :],
                w8t[p // PCHUNK][:, p % PCHUNK, :, c0:c0 + 512],
                start=(p == 0), stop=(p == NPAIR - 1),
                perf_mode=DR,
            )

    def evict(t, tt, ps, last):
        row0 = t * TOK_TILE + tt * P
        osb = opool.tile([P, NSH], f16, tag="osb")
        if not last:
            for c0, cw, pst in ((0, NW16, ps[0]), (NW16, 512, ps[1]),
                                (NW16 + 512, 512, ps[2])):
                nc.vector.tensor_add(osb[:, c0:c0 + cw], pst[:],
                                     bias_b[:, c0:c0 + cw])
            nc.gpsimd.dma_start(out=out[row0:row0 + P, :], in_=osb[:])
            return
        # Final subtile: first two chunks drain while the sliced last DR
        # chain still streams; the 512-col tail goes out in 4x128 slices
        # alternating across both HWDGE rings so only ~1us of add+store
        # trails the last matmul.
        for c0, cw, pst in ((0, NW16, ps[0]), (NW16, 512, ps[1])):
            nc.vector.tensor_add(osb[:, c0:c0 + cw], pst[:], bias_b[:, c0:c0 + cw])
            nc.scalar.dma_start(out=out[row0:row0 + P, c0:c0 + cw],
                                in_=osb[:, c0:c0 + cw])
        for s in range(4):
            c0 = NW16 + 512 + 128 * s
            nc.vector.tensor_add(osb[:, c0:c0 + 128],
                                 ps[2][:, 128 * s:128 * s + 128],
                                 bias_b[:, c0:c0 + 128])
            eng = (nc.sync, nc.scalar)[s % 2]
            eng.dma_start(out=out[row0:row0 + P, c0:c0 + 128],
                          in_=osb[:, c0:c0 + 128])

    def alloc_ps():
        return [ppool.tile([P, NW16], f32, tag="ps", name="ps0"),
                ppool.tile([P, 512], f32, tag="ps", name="ps1"),
                ppool.tile([P, 512], f32, tag="ps", name="ps2")]

    # --- t0 pair (A,B): supply-paced chunk-major ------------------------
    # The first pair is DMA-supply-bound (all of W plus 4MB of x must
    # stream in at ~370GB/s aggregate while the PE wants to run).  Both
    # fp16 chains run pair-chunk-major (A then B per 4-group w16 chunk, x
    # in NG-quarters) and both DR chains pair-chunk-major per wf8 chunk,
    # so each arriving chunk unlocks ~1.2-1.7us of work and no single
    # stall exceeds the ~3us HAM downshift threshold.
    def cast_x8_q(qs):
        x8c = x8pool.tile([P, NG, P], f8, tag="x8c")
        for j in range(4):
            nc.vector.tensor_copy(x8c[:, 8 * j:8 * j + 8, :], qs[j][:])
        return x8c

    x8A, x8B = cast_x8_q(xq[0]), cast_x8_q(xq[1])
    psA, psB = alloc_ps(), alloc_ps()
    for c in range(NG // GCHUNK):
        for tt in (0, 1):
            ps0 = (psA, psB)[tt][0]
            for g4 in range(GCHUNK):
                g = GCHUNK * c + g4
                j = g // 8
                nc.tensor.matmul(
                    ps0[:, :], xq[tt][j][:, g - 8 * j, :],
                    w16t[c][:, g4, :],
                    start=(g == 0), stop=(g == NG - 1),
                )
        if c < NG // GCHUNK - 1:
            # keep the supply-paced gap under the ~1.9us HAM downshift
            # threshold while the next w16 chunk streams in (the scalar
            # ring's ~3.4us/chunk cadence leaves ~2us holes from c3 on)
            dummies(1 if c < 3 else 2)


    for pc in range(NPAIR // PCHUNK):
        for tt in (0, 1):
            x8c = (x8A, x8B)[tt]
            ps1, ps2 = (psA, psB)[tt][1], (psA, psB)[tt][2]
            for pp in range(PCHUNK * pc, PCHUNK * (pc + 1)):
                for c0, pst in ((0, ps1), (512, ps2)):
                    nc.tensor.matmul(
                        pst[:, :], x8c[:, 2 * pp:2 * pp + 2, :],
                        w8t[pc][:, pp % PCHUNK, :, c0:c0 + 512],
                        start=(pp == 0), stop=(pp == NPAIR - 1),
                        perf_mode=DR,
                    )
    evict(0, 0, psA, False)
    evict(0, 1, psB, False)

    # --- t0 pair (C,D): m-tile-major ------------------------------------
    # x02/x03 land late (~38/46us) on the saturated rings, so C runs to
    # completion (its x tile arrives first) before D touches x03; the two
    # extra fp16<->DR transitions cost ~64ns each.
    x8C = cast_x8(cur23[0])
    psC = alloc_ps()
    fp16_chain(cur23[0], psC[0])
    nxt = [load_x_tt(1, s) for s in range(TOK_TILE // P)]
    dr_chains(x8C, psC[1], psC[2],
              [(p, ci) for p in range(NPAIR) for ci in (0, 1)])
    evict(0, 2, psC, False)
    x8D = cast_x8(cur23[1])
    psD = alloc_ps()
    fp16_chain(cur23[1], psD[0])
    dr_chains(x8D, psD[1], psD[2],
              [(p, ci) for p in range(NPAIR) for ci in (0, 1)])
    evict(0, 3, psD, False)

    # --- t=1..7: steady-state pairs -------------------------------------
    # Subtiles run in PAIRS — fp16(A), fp16(B), DR(A), DR(B).  Six PSUM
    # banks hold exactly one pair's three chains x2.
    cur = nxt
    for t in range(1, NT):
        for pr in range(2):
            ttA, ttB = 2 * pr, 2 * pr + 1
            # Prefetch next t's x tiles during the first pair.
            if pr == 0 and t + 1 < NT:
                nxt = [load_x_tt(t + 1, s) for s in range(TOK_TILE // P)]
            x8A, x8B = cast_x8(cur[ttA]), cast_x8(cur[ttB])
            psA, psB = alloc_ps(), alloc_ps()
            last = (t == NT - 1) and (pr == 1)
            fp16_chain(cur[ttA], psA[0])
            fp16_chain(cur[ttB], psB[0])
            dr_chains(x8A, psA[1], psA[2],
                      [(p, ci) for p in range(NPAIR) for ci in (0, 1)])
            evict(t, ttA, psA, False)
            if not last:
                dr_chains(x8B, psB[1], psB[2],
                          [(p, ci) for p in range(NPAIR) for ci in (0, 1)])
            else:
                # Final subtile: sequential DR chains so ps1 drains through
                # DVE/DMA while ps2 still streams — shortens the tail.
                # (Column-slicing the ps2 CHAIN is unsound: interleaved
                # accumulation chains in one PSUM bank clobber each other's
                # partials via the start_tensor_calc bank clear; only the
                # EVICTION is sliced, in evict().)
                dr_chains(x8B, psB[1], psB[2], [(p, 0) for p in range(NPAIR)])
                dr_chains(x8B, psB[1], psB[2], [(p, 1) for p in range(NPAIR)])
            evict(t, ttB, psB, last)
        cur = nxt if t + 1 < NT else None


# Engine-compute ISA structs encode very few sync-wait slots (the DVE
# tensor ops hold only one); walrus codegen hard-fails on excess.  Tile's
# scheduler may attach several waits to one instruction, so after
# scheduling we move the surplus onto same-engine Drain instructions
# inserted immediately before (the engine stalls there instead — same
# semantics, and drains legally carry many waits).
_WAIT_LIMITED = {
    "InstTensorTensor", "InstTensorScalarPtr", "InstTensorCopy",
    "InstTensorReduce", "InstMemset", "InstActivation", "InstIota",
    "InstMatmult", "InstLdweights", "InstBNStats", "InstBNStatsAggregate",
    "InstDrain", "InstDMACopy",
}


def _dedupe_ldweights(nc):
    """Delete back-to-back redundant LDWEIGHTS.

    The two column-chunk matmuls of each DoubleRow pair share one
    stationary tile, but bass emits an Ldweights per matmul and the
    compiler-side dedup (--enable-ldw-opt) is disabled.  Reloading
    identical weights is idempotent, so an Ldweights whose source AP
    equals the previous one on the PE stream — with only matmuls in
    between, no sync waits and no sem updates of its own — can be
    dropped.  W tiles are written once and never recycled, and x-tile
    slot reuse is gated on the matmuls' sem increments (Ldweights never
    increments), so sem bookkeeping is unchanged.
    """
    removed = 0
    for fn in nc.m.functions:
        for bb in fn.blocks:
            newl = []
            prev_ldw_key = None
            for inst in bb.instructions:
                t = type(inst).__name__
                eng = str(inst.engine)
                if eng == "EngineType.PE":
                    if t == "InstLdweights":
                        si = inst.sync_info
                        has_sync = si is not None and (si.on_wait or si.on_update)
                        key = str(inst.ins)
                        if key == prev_ldw_key and not has_sync:
                            removed += 1
                            continue
                        prev_ldw_key = key
                    elif t != "InstMatmult":
                        prev_ldw_key = None
                newl.append(inst)
            bb.instructions[:] = newl
    return removed


def _reduce_waits(nc):
    """Drop transitively-implied semaphore waits.

    A wait (sem, v) on instruction X is redundant when another wait on X
    targets a producer whose vector clock already covers (sem, v), when
    X's own proc has already observed it, or when the sem belongs to X's
    own in-order proc (same-FIFO dominance).  Two phases: build complete
    per-sem producer vector clocks (block list order is per-proc
    consistent; cross-proc misses only make the result conservative),
    then prune using the final maps.  Only 'sem-ge-imm' waits and
    incrementing ('sem-inc'/'sem-add-imm') updates participate; any other
    update mode invalidates that sem's history.
    """
    INC = ("sem-inc", "sem-add-imm")
    for fn in nc.m.functions:
        insts = [inst for bb in fn.blocks for inst in bb.instructions]

        def params(inst):
            si = inst.sync_info
            waits = list(si.on_wait) if si is not None and si.on_wait else []
            ups = list(si.on_update) if si is not None and si.on_update else []
            proc = (str(inst.engine), getattr(inst, "bass_scheduled_proc", None))
            return si, waits, ups, proc

        def wait_vc(prodvc, w):
            if w.wait_mode != "sem-ge-imm" or w.wait_reg is not None:
                return None
            for cv, vc in prodvc.get(w.id, []):
                if cv >= w.wait_value:
                    return vc
            return None

        def build_maps(lookup_prodvc):
            cum, prodvc, procvc, updaters, obsvc = {}, {}, {}, {}, {}
            for inst in insts:
                si, waits, ups, proc = params(inst)
                myvc = dict(procvc.get(proc, {}))
                for w in waits:
                    if w.wait_mode == "sem-ge-imm" and w.wait_reg is None:
                        vc = wait_vc(lookup_prodvc if lookup_prodvc is not None
                                     else prodvc, w)
                        if vc is not None:
                            for k, v in vc.items():
                                if myvc.get(k, 0) < v:
                                    myvc[k] = v
                        if myvc.get(w.id, 0) < w.wait_value:
                            myvc[w.id] = w.wait_value
                procvc[proc] = myvc
                obsvc[id(inst)] = myvc
                for u in ups:
                    if u.update_mode in INC and u.update_reg is None:
                        cum[u.id] = cum.get(u.id, 0) + u.update_value
                        updaters.setdefault(u.id, set()).add(proc)
                        snap = dict(myvc)
                        snap[u.id] = cum[u.id]
                        prodvc.setdefault(u.id, []).append((cum[u.id], snap))
                    else:
                        cum.pop(u.id, None)
                        prodvc.pop(u.id, None)
                        updaters[u.id] = {object()}
            return cum, prodvc, updaters, obsvc

        # Pass 1 builds conservative clocks; pass 2 rebuilds them resolving
        # forward references through pass 1's complete producer map.
        _, prodvc, _, _ = build_maps(None)
        _, prodvc, _, _ = build_maps(prodvc)

        # Prune with the final maps, tracking per-proc observation and
        # per-proc cumulative sem updates in list order.
        cum, procvc, updaters = {}, {}, {}
        for inst in insts:
            si, waits, ups, proc = params(inst)
            myvc = dict(procvc.get(proc, {}))
            if len(waits) > 1:
                vcs = [wait_vc(prodvc, w) for w in waits]
                keep_ws = []
                for i, w in enumerate(waits):
                    if w.wait_mode == "sem-ge-imm" and w.wait_reg is None:
                        if myvc.get(w.id, 0) >= w.wait_value:
                            continue
                        if (updaters.get(w.id) == {proc}
                                and cum.get(w.id, 0) >= w.wait_value):
                            continue
                        implied = any(
                            j != i and vcs[j] is not None
                            and vcs[j].get(w.id, 0) >= w.wait_value
                            for j in range(len(waits)))
                        if implied:
                            continue
                    keep_ws.append(w)
                if len(keep_ws) != len(waits):
                    inst.sync_info = mybir.SyncInfo(on_wait=keep_ws, on_update=ups)
                    waits = keep_ws
            for w in waits:
                if w.wait_mode == "sem-ge-imm" and w.wait_reg is None:
                    vc = wait_vc(prodvc, w)
                    if vc is not None:
                        for k, v in vc.items():
                            if myvc.get(k, 0) < v:
                                myvc[k] = v
                    if myvc.get(w.id, 0) < w.wait_value:
                        myvc[w.id] = w.wait_value
            procvc[proc] = myvc
            for u in ups:
                if u.update_mode in INC and u.update_reg is None:
                    cum[u.id] = cum.get(u.id, 0) + u.update_value
                    updaters.setdefault(u.id, set()).add(proc)
                else:
                    cum.pop(u.id, None)
                    updaters[u.id] = {object()}


def _legalize_waits(nc, keep=1, drain_cap=1):
    for fn in nc.m.functions:
        for bb in fn.blocks:
            newl = []
            for inst in bb.instructions:
                si = inst.sync_info
                waits = list(si.on_wait) if si is not None and si.on_wait else []
                if type(inst).__name__ in _WAIT_LIMITED and len(waits) > keep:
                    extra, kept = waits[:-keep], waits[-keep:]
                    for i in range(0, len(extra), drain_cap):
                        d = mybir.InstDrain(name=f"{inst.name}-wsplit{i}")
                        d.engine = inst.engine
                        d.sync_info = mybir.SyncInfo(
                            on_wait=extra[i : i + drain_cap], on_update=[])
                        newl.append(d)
                    inst.sync_info = mybir.SyncInfo(
                        on_wait=kept,
                        on_update=list(si.on_update) if si.on_update else [])
                newl.append(inst)
            bb.instructions[:] = newl
    return


_NC_CACHE = None


def build_nc(legalize=True):
    global _NC_CACHE
    if _NC_CACHE is not None:
        return _NC_CACHE
    nc = bass.Bass("TRN2", target_bir_lowering=False, debug=False)
    xt = nc.dram_tensor("xt", [NT, TOK_TILE // P, P, NG, P], f16, kind="ExternalInput").ap()
    w16 = nc.dram_tensor("w16", [P, NG, NW16], f16, kind="ExternalInput").ap()
    wf8 = nc.dram_tensor("wf8", [P, NPAIR, 2, NW8], f8, kind="ExternalInput").ap()
    biasv = nc.dram_tensor("biasv", [NSH], f32, kind="ExternalInput").ap()
    out = nc.dram_tensor("out", [M, NSH], f16, kind="ExternalOutput").ap()
    with tile.TileContext(nc) as tc:
        with ExitStack() as ctx:
            _kernel_body(ctx, tc, out, xt, w16, wf8, biasv)
    if legalize:
        _dedupe_ldweights(nc)
        _reduce_waits(nc)
        _legalize_waits(nc)
        _NC_CACHE = nc
    return nc


def prep_in_maps(inputs):
    """Host-side shard/layout prep.  Returns (in_maps, perm)."""
    x = np.asarray(inputs["x"], np.float32)
    hw = np.asarray(inputs["high_prec_weight"])
    hs = np.asarray(inputs["high_prec_scales"], np.float32)
    lw = np.asarray(inputs["low_prec_weight"])
    ls1 = np.asarray(inputs["low_prec_scales"], np.float32)
    ls2 = np.asarray(inputs["low_prec_scales2"], np.float32)
    lz = np.asarray(inputs["low_prec_zeros"], np.float32)
    perm = np.asarray(inputs["col_indices_inv"]).astype(np.int64)
    bias = np.asarray(inputs["bias"], np.float32)

    x2 = x.reshape(M, K)
    # x^T tiled per 128-token subtile:
    #   xt[t, s, p, g, m] = x[t*512 + s*128 + m, g*128 + p].
    xr = x2.reshape(NT, TOK_TILE // P, P, NG, P)
    xt = np.ascontiguousarray(xr.astype(np.float16).transpose(0, 1, 4, 3, 2))

    # Dequantized weights in concat order.
    w_high = (hw.astype(np.float32).reshape(NG, GROUP, N_HIGH)
              * hs[:, None, :]).reshape(K, N_HIGH)
    w_low = (lw.astype(np.float32) - lz) * ls1 * ls2

    bias_cat = np.empty(OUT_F, np.float32)
    bias_cat[perm] = bias  # bias_cat[col_inv[j]] = bias[j]

    in_maps = []
    for c in range(NCORES):
        hsl = slice(c * NH, (c + 1) * NH)
        lsl0 = c * NL            # stray 8 low cols -> fp16 path
        lsl8 = slice(c * NL + 8, (c + 1) * NL)  # 1024 low cols -> fp8 path
        w16c = np.concatenate(
            [w_high[:, hsl], w_low[:, lsl0:lsl0 + 8]], axis=1).astype(np.float16)
        w8c = w_low[:, lsl8].astype(ml_dtypes.float8_e4m3)
        biasv = np.concatenate(
            [bias_cat[c * NH:(c + 1) * NH],
             bias_cat[N_HIGH + c * NL:N_HIGH + (c + 1) * NL]]).astype(np.float32)
        in_maps.append({
            "xt": xt,
            "w16": np.ascontiguousarray(
                w16c.reshape(NG, P, NW16).transpose(1, 0, 2)),
            "wf8": np.ascontiguousarray(
                w8c.reshape(NPAIR, 2, P, NW8).transpose(2, 0, 1, 3)),
            "biasv": np.ascontiguousarray(biasv),
        })
    return in_maps, perm


def gather_output(results, perm):
    cat = np.empty((M, OUT_F), np.float32)
    for c in range(NCORES):
        o = results[c]["out"]
        cat[:, c * NH:(c + 1) * NH] = o[:, :NH]
        cat[:, N_HIGH + c * NL:N_HIGH + (c + 1) * NL] = o[:, NH:]
    final = np.take(cat, perm, axis=1)
    return np.ascontiguousarray(final.reshape(B, S, OUT_F).astype(np.float32))


def run(inputs, **spmd_kwargs):
    """Run on hardware; returns (output, BassKernelResults)."""
    nc = build_nc()
    in_maps, perm = prep_in_maps(inputs)
    res = run_bass_kernel_spmd(nc, in_maps, list(range(NCORES)), **spmd_kwargs)
    return gather_output(res.results, perm), res


def kernel(**inputs) -> np.ndarray:
    out, _ = run(inputs)
    return out



# revision 37
# speedup vs baseline: 1.1914x; 1.0019x over previous
"""CPRLinearMultiPrecision kernel for 8 TRN2 NeuronCores — fp8 DoubleRow.

The reference absmax is dominated by the high-precision columns (out std
~55 vs ~6 for the low columns), so the 2e-2 relative-error budget leaves
the low columns ~5 absolute error headroom.  That pays for an fp8
matmul on the low 75% of columns:

  * low cols:  W8 = e4m3((q - z[k]) * s1[k] * s2[c]) prepared on host,
    x8 = e4m3(x).  PE runs perf_mode=DoubleRow: both operands
    [128, 2, N] fp8, contracting TWO 128-row K-groups per column-cycle
    (~1.13 cyc/col vs 1.0 for fp16 but half the matmuls).  Measured
    numpy error: max |err| ~2.0 « 6.5 budget (rel ~6e-3 total).
  * high cols: exact int8*scale structure needs >4 significant bits, and
    a 2-pass fp8 split only ties fp16, so they stay an fp16 matmul with
    host-dequantized W.

Every core gets an equal mix (column-parallel, rebalanced): 344 high
cols + 8 stray low cols as the fp16 path (352 = one PSUM chunk) + 1024
low cols as the fp8 path (2 x 512 chunks).  Per 128-token subtile the
PE runs 32 fp16 matmuls (N=352, 149ns) + 16 DoubleRow pairs (2 matmuls
of N=512, 216ns — no measured DR streaming penalty), ~11.9us; subtiles
run in pairs (fp16 A, fp16 B, DR A, DR B — six PSUM banks hold exactly
one pair) to halve the fp16<->DR phase transitions; the fp8 x tile is
cast on-device by the DVE rather than DMA'd, and the DVE also drains
psum+bias -> fp16 output tiles.

t=0 is supply-bound: all of W (7.1MB) plus 4MB of x must land at the
~345GB/s aggregate HBM cap before the PE has anything to chew on, and
the HAM clock state machine makes this expensive twice over — the PE
boots at K=4/8 (half clock), upshifts only after ~3.2us of GAPLESS
matmul activity, and downshifts again on any ~2us idle.  Countermeasures
(worth ~7us at 2.4GHz vs the plain EDF prologue):
  * 32 dependency-free garbage matmuls right after the preamble warm the
    array to K=8/8 by ~13us and bridge the PE to the first real supply
    (the rings only start flowing at ~9.1/10.6/12.3us and share the cap
    roughly 85/105/160 GB/s once all active);
  * the first pair runs pair-chunk-major (A then B per 4-group w16
    chunk, x in NG-quarters; per-chunk stalls padded with 1-2 garbage
    matmuls stay under the downshift threshold), then chunk-major DR;
  * the second pair (C,D) is m-tile-major since x03 lands ~8us after
    x02; steady-state pairs then run exactly at the 23.36us streaming
    floor (measured 23.38).
The final subtile's last 512-col chain is evicted in 4x128-col slices
across both HWDGE rings so only ~1.3us of add+store rides the tail.

Host side: dequantize/quantize W, fold s1/s2/z into the fp8 codes (bias
is applied on device); gather = concat + column permutation.

Measured on 8 TRN2 cores: ~409us HW exec at 2.4GHz (vs 411.3us for the
plain-EDF baseline; PE streaming floor 373.8us + ~8us preamble+ramp +
~5us counted tail), rel err ~5.5e-3 vs the fp32 reference (budget
2e-2).  Beware run-to-run P0 downclock (PE upshifts to 2.0GHz instead
of 2.4): check MATMUL start-to-start is 149/216ns before comparing
timings.

Post-scheduling passes (unchanged from the fp16 baseline):
_dedupe_ldweights drops back-to-back LDWEIGHTS reloading an unchanged
stationary tile (the compiler-side --enable-ldw-opt is disabled);
_reduce_waits prunes transitively-implied semaphore waits; and
_legalize_waits moves surplus waits onto same-engine Drains (engine ISA
structs encode a single sync-wait slot).
"""

import numpy as np
import ml_dtypes
from contextlib import ExitStack

import concourse.bass as bass
import concourse.tile as tile
from concourse import mybir
from concourse.bass_utils import run_bass_kernel_spmd

# Problem shapes (hardcoded; kernel.py must be self-contained).
B, S = 2, 2048
M = B * S              # 4096 tokens
K = 4096               # in features
OUT_F = 11008
N_HIGH = 2752
N_LOW = OUT_F - N_HIGH  # 8256
GROUP = 128
NG = K // GROUP        # 32 K-groups
NPAIR = NG // 2        # 16 DoubleRow group-pairs
NCORES = 8
NH = N_HIGH // NCORES  # 344 high cols per core
NL = N_LOW // NCORES   # 1032 low cols per core
NW16 = NH + 8          # fp16-path cols per core (344 high + 8 stray low)
NW8 = NL - 8           # fp8-path cols per core (1024)
NSH = NW16 + NW8       # 1376 output cols per core
P = 128
TOK_TILE = 512
NT = M // TOK_TILE     # 8
GCHUNK = 4             # fp16 W groups per DMA chunk
PCHUNK = 2             # fp8 W pairs per DMA chunk

f8 = mybir.dt.float8e4
f16 = mybir.dt.float16
f32 = mybir.dt.float32

DR = mybir.MatmulPerfMode.DoubleRow


def _bcast(ap_1d, parts):
    """Partition-broadcast AP: replicate a 1-D DRAM row across `parts` partitions."""
    return bass.AP(tensor=ap_1d.tensor, offset=ap_1d.offset, ap=[[0, parts]] + ap_1d.ap)


def _kernel_body(ctx, tc, out, xt, w16, wf8, biasv):
    nc = tc.nc
    const = ctx.enter_context(tc.tile_pool(name="const", bufs=1))
    w16pool = ctx.enter_context(tc.tile_pool(name="w16", bufs=NG // GCHUNK))
    w8pool = ctx.enter_context(tc.tile_pool(name="w8", bufs=NPAIR // PCHUNK))
    xpool = ctx.enter_context(tc.tile_pool(name="x", bufs=8))
    xqpool = ctx.enter_context(tc.tile_pool(name="xq", bufs=8))
    x8pool = ctx.enter_context(tc.tile_pool(name="x8", bufs=8))
    opool = ctx.enter_context(tc.tile_pool(name="o", bufs=4))
    ppool = ctx.enter_context(tc.tile_pool(name="p", bufs=6, space="PSUM"))

    # t=0 needs ALL of W (7.1MB) plus its tt=0 x slices within its first
    # ~12us of PE work, and each DMA ring sustains only ~125GB/s draining
    # FIFO — so x is tiled per 128-token subtile (1.5MB/subtile instead of
    # 6.3MB/t upfront) and the t0-critical transfers are spread across all
    # three rings (sync/scalar HWDGE + gpsimd SWDGE) in PE consumption
    # order (earliest-deadline-first).
    w16t = [w16pool.tile([P, GCHUNK, NW16], f16, tag="w16", name=f"w16c{i}")
            for i in range(NG // GCHUNK)]
    w8t = [w8pool.tile([P, PCHUNK, 2, NW8], f8, tag="w8", name=f"w8c{i}")
           for i in range(NPAIR // PCHUNK)]
    garb = const.tile([P, 512], f16)
    bias_b = const.tile([P, NSH], f32)

    def load_x_tt(t, tt, eng=None):
        xc = xpool.tile([P, NG, P], f16, tag="xc")
        (eng or nc.sync).dma_start(out=xc[:], in_=xt[t, tt, :, :, :])
        return xc

    def cast_x8(xc):
        # fp8 copy of the x subtile for the DoubleRow path — derived
        # on-device (DVE) instead of a second HBM stream, keeping the
        # ring-bound t0 prologue lean.
        x8c = x8pool.tile([P, NG, P], f8, tag="x8c")
        nc.vector.tensor_copy(x8c[:], xc[:])
        return x8c

    # --- PE clock warm-up -------------------------------------------------
    # The HAM state machine starts the PE at K=4/8 (half clock) and only
    # upshifts after ~3.2us of GAPLESS matmul activity; any >=0.5us idle
    # resets the accumulator, and t0 is full of supply stalls — on the
    # baseline the array stayed at half clock until t~42us.  A burst of
    # dependency-free garbage matmuls right after the framework preamble
    # (PE idle until ~18us otherwise: DMA rings only start flowing at
    # ~9.6us) upshifts the clock by ~12us, before the first real matmul.
    nc.vector.memset(garb[:], 1.0)
    dummy_ps = ppool.tile([P, 512], f32, tag="ps", name="warm")

    def dummies(n):
        for _ in range(n):
            nc.tensor.matmul(dummy_ps[:, :], garb[:, :P], garb[:, :],
                             start=True, stop=True)

    dummies(32)

    # Prologue, spread EDF-style over the three rings in PE consumption
    # order.  The fp16 phase of the first pair runs pair-chunk-major, so
    # its critical supply (w16c0 + x(0,0)/x(0,1) in NG-quarters) leads the
    # fast-starting sync ring; w16c1..7 ride scalar; wf8 is split so the
    # chunk-major DR(A,B) phase never waits:
    #   sync:   w16c0, x00/x01 quarters (interleaved), wf8c5-6, x02, x03
    #   scalar: w16c1..4, bias, w16c5..7
    #   gpsimd: wf8c0..4, wf8c7
    def load_w8(i, eng):
        eng.dma_start(out=w8t[i][:], in_=wf8[:, i * PCHUNK:(i + 1) * PCHUNK, :, :])

    # EDF split across the three rings using their measured shares when all
    # are active (sync ~85GB/s, scalar ~105, gpsimd-SWDGE ~160; the SWDGE
    # ring starts ~3.5us later but drains fastest).  Deadline order is the
    # PE's: w16c+x-quarters paced ~1.4us/chunk from ~15us, then wf8 chunks
    # ~2us apart from ~29us, then x02/x03/bias for the C,D pair.
    def load_w16(i, eng):
        eng.dma_start(out=w16t[i][:], in_=w16[:, i * GCHUNK:(i + 1) * GCHUNK, :])

    def load_xq(tt, j, eng):
        q = xqpool.tile([P, 8, P], f16, tag="xq")
        eng.dma_start(out=q[:], in_=xt[0, tt, :, 8 * j:8 * j + 8, :])
        return q

    xq = [[None] * 4 for _ in range(2)]  # xq[tt][j]: groups 8j..8j+7 of x(0,tt)
    # sync: first-matmul critical path (w16c0 + all x quarters), then the
    # late wf8 chunks and x03
    load_w16(0, nc.sync)
    for j in range(4):
        xq[0][j] = load_xq(0, j, nc.sync)
        xq[1][j] = load_xq(1, j, nc.sync)
    for i in (5, 6):
        load_w8(i, nc.sync)
    x02 = load_x_tt(0, 2, nc.sync)
    x03 = load_x_tt(0, 3, nc.sync)
    # scalar: the rest of w16, bias mid-stream
    for i in (1, 2, 3, 4):
        load_w16(i, nc.scalar)
    nc.scalar.dma_start(out=bias_b[:], in_=_bcast(biasv[:], P))
    for i in (5, 6, 7):
        load_w16(i, nc.scalar)
    # gpsimd (fast once its ~3.5us SWDGE spin-up passes): the early wf8
    # chunks (consumed from ~28us).  NOTE: shifting MORE onto the SWDGE
    # ring (all 8 wf8 chunks, or w16/x tiles) measured consistently worse
    # — its effective rate degrades with queue depth, and the HWDGE/SWDGE
    # share split drifts ±30% run-to-run, so this EDF split was tuned
    # empirically, not from a rate model.
    for i in (0, 1, 2, 3, 4, 7):
        load_w8(i, nc.gpsimd)
    cur23 = [x02, x03]

    def fp16_chain(xc, ps0):
        # stationary x [128k x 128tok], moving W16 [128k x 352]
        for g in range(NG):
            nc.tensor.matmul(
                ps0[:, :], xc[:, g, :],
                w16t[g // GCHUNK][:, g % GCHUNK, :],
                start=(g == 0), stop=(g == NG - 1),
            )

    def dr_chains(x8c, ps1, ps2, order):
        # fp8 DoubleRow: both operands [128, 2, free]; contracts groups
        # (2p, 2p+1) per column-cycle.
        for p, ci in order:
            c0, pst = ((0, ps1), (512, ps2))[ci]
            nc.tensor.matmul(
                pst[:, :], x8c[:, 2 * p:2 * p + 2, :],
                w8t[p // PCHUNK][:, p % PCHUNK, :, c0:c0 + 512],
                start=(p == 0), stop=(p == NPAIR - 1),
                perf_mode=DR,
            )

    def evict(t, tt, ps, last):
        row0 = t * TOK_TILE + tt * P
        osb = opool.tile([P, NSH], f16, tag="osb")
        if not last:
            for c0, cw, pst in ((0, NW16, ps[0]), (NW16, 512, ps[1]),
                                (NW16 + 512, 512, ps[2])):
                nc.vector.tensor_add(osb[:, c0:c0 + cw], pst[:],
                                     bias_b[:, c0:c0 + cw])
            # Stores ride the scalar HWDGE ring, which is idle after the
            # t0 w16/bias loads: on the gpsimd SWDGE ring (~29GB/s
            # effective) the 31x352KB output stream needs ~24.2us of queue
            # time per 23.38us pair, so the queue falls steadily behind and
            # the end-of-kernel drain waits out the backlog.
            nc.scalar.dma_start(out=out[row0:row0 + P, :], in_=osb[:])
            return
        # Final subtile: first two chunks drain while the sliced last DR
        # chain still streams; the 512-col tail goes out in 4x128 slices
        # alternating across both HWDGE rings so only ~1us of add+store
        # trails the last matmul.
        for c0, cw, pst in ((0, NW16, ps[0]), (NW16, 512, ps[1])):
            nc.vector.tensor_add(osb[:, c0:c0 + cw], pst[:], bias_b[:, c0:c0 + cw])
            nc.scalar.dma_start(out=out[row0:row0 + P, c0:c0 + cw],
                                in_=osb[:, c0:c0 + cw])
        for s in range(4):
            c0 = NW16 + 512 + 128 * s
            nc.vector.tensor_add(osb[:, c0:c0 + 128],
                                 ps[2][:, 128 * s:128 * s + 128],
                                 bias_b[:, c0:c0 + 128])
            eng = (nc.sync, nc.scalar)[s % 2]
            eng.dma_start(out=out[row0:row0 + P, c0:c0 + 128],
                          in_=osb[:, c0:c0 + 128])

    def alloc_ps():
        return [ppool.tile([P, NW16], f32, tag="ps", name="ps0"),
                ppool.tile([P, 512], f32, tag="ps", name="ps1"),
                ppool.tile([P, 512], f32, tag="ps", name="ps2")]

    # --- t0 pair (A,B): supply-paced chunk-major ------------------------
    # The first pair is DMA-supply-bound (all of W plus 4MB of x must
    # stream in at ~370GB/s aggregate while the PE wants to run).  Both
    # fp16 chains run pair-chunk-major (A then B per 4-group w16 chunk, x
    # in NG-quarters) and both DR chains pair-chunk-major per wf8 chunk,
    # so each arriving chunk unlocks ~1.2-1.7us of work and no single
    # stall exceeds the ~3us HAM downshift threshold.
    def cast_x8_q(qs):
        x8c = x8pool.tile([P, NG, P], f8, tag="x8c")
        for j in range(4):
            nc.vector.tensor_copy(x8c[:, 8 * j:8 * j + 8, :], qs[j][:])
        return x8c

    x8A, x8B = cast_x8_q(xq[0]), cast_x8_q(xq[1])
    psA, psB = alloc_ps(), alloc_ps()
    for c in range(NG // GCHUNK):
        for tt in (0, 1):
            ps0 = (psA, psB)[tt][0]
            for g4 in range(GCHUNK):
                g = GCHUNK * c + g4
                j = g // 8
                nc.tensor.matmul(
                    ps0[:, :], xq[tt][j][:, g - 8 * j, :],
                    w16t[c][:, g4, :],
                    start=(g == 0), stop=(g == NG - 1),
                )
        if c < NG // GCHUNK - 1:
            # keep the supply-paced gap under the ~1.9us HAM downshift
            # threshold while the next w16 chunk streams in (the scalar
            # ring's ~3.4us/chunk cadence leaves ~2us holes from c3 on)
            dummies(1 if c < 3 else 2)


    for pc in range(NPAIR // PCHUNK):
        for tt in (0, 1):
            x8c = (x8A, x8B)[tt]
            ps1, ps2 = (psA, psB)[tt][1], (psA, psB)[tt][2]
            for pp in range(PCHUNK * pc, PCHUNK * (pc + 1)):
                for c0, pst in ((0, ps1), (512, ps2)):
                    nc.tensor.matmul(
                        pst[:, :], x8c[:, 2 * pp:2 * pp + 2, :],
                        w8t[pc][:, pp % PCHUNK, :, c0:c0 + 512],
                        start=(pp == 0), stop=(pp == NPAIR - 1),
                        perf_mode=DR,
                    )
    evict(0, 0, psA, False)
    evict(0, 1, psB, False)

    # --- t0 pair (C,D): m-tile-major ------------------------------------
    # x02/x03 land late (~38/46us) on the saturated rings, so C runs to
    # completion (its x tile arrives first) before D touches x03; the two
    # extra fp16<->DR transitions cost ~64ns each.
    x8C = cast_x8(cur23[0])
    psC = alloc_ps()
    fp16_chain(cur23[0], psC[0])
    nxt = [load_x_tt(1, s) for s in range(TOK_TILE // P)]
    dr_chains(x8C, psC[1], psC[2],
              [(p, ci) for p in range(NPAIR) for ci in (0, 1)])
    evict(0, 2, psC, False)
    x8D = cast_x8(cur23[1])
    psD = alloc_ps()
    fp16_chain(cur23[1], psD[0])
    dr_chains(x8D, psD[1], psD[2],
              [(p, ci) for p in range(NPAIR) for ci in (0, 1)])
    evict(0, 3, psD, False)

    # --- t=1..7: steady-state pairs -------------------------------------
    # Subtiles run in PAIRS — fp16(A), fp16(B), DR(A), DR(B).  Six PSUM
    # banks hold exactly one pair's three chains x2.
    cur = nxt
    for t in range(1, NT):
        for pr in range(2):
            ttA, ttB = 2 * pr, 2 * pr + 1
            # Prefetch next t's x tiles during the first pair.
            if pr == 0 and t + 1 < NT:
                nxt = [load_x_tt(t + 1, s) for s in range(TOK_TILE // P)]
            x8A, x8B = cast_x8(cur[ttA]), cast_x8(cur[ttB])
            psA, psB = alloc_ps(), alloc_ps()
            last = (t == NT - 1) and (pr == 1)
            fp16_chain(cur[ttA], psA[0])
            fp16_chain(cur[ttB], psB[0])
            dr_chains(x8A, psA[1], psA[2],
                      [(p, ci) for p in range(NPAIR) for ci in (0, 1)])
            evict(t, ttA, psA, False)
            if not last:
                dr_chains(x8B, psB[1], psB[2],
                          [(p, ci) for p in range(NPAIR) for ci in (0, 1)])
            else:
                # Final subtile: sequential DR chains so ps1 drains through
                # DVE/DMA while ps2 still streams — shortens the tail.
                # (Column-slicing the ps2 CHAIN is unsound: interleaved
                # accumulation chains in one PSUM bank clobber each other's
                # partials via the start_tensor_calc bank clear; only the
                # EVICTION is sliced, in evict().)
                dr_chains(x8B, psB[1], psB[2], [(p, 0) for p in range(NPAIR)])
                dr_chains(x8B, psB[1], psB[2], [(p, 1) for p in range(NPAIR)])
            evict(t, ttB, psB, last)
        cur = nxt if t + 1 < NT else None


# Engine-compute ISA structs encode very few sync-wait slots (the DVE
# tensor ops hold only one); walrus codegen hard-fails on excess.  Tile's
# scheduler may attach several waits to one instruction, so after
# scheduling we move the surplus onto same-engine Drain instructions
# inserted immediately before (the engine stalls there instead — same
# semantics, and drains legally carry many waits).
_WAIT_LIMITED = {
    "InstTensorTensor", "InstTensorScalarPtr", "InstTensorCopy",
    "InstTensorReduce", "InstMemset", "InstActivation", "InstIota",
    "InstMatmult", "InstLdweights", "InstBNStats", "InstBNStatsAggregate",
    "InstDrain", "InstDMACopy",
}


def _dedupe_ldweights(nc):
    """Delete back-to-back redundant LDWEIGHTS.

    The two column-chunk matmuls of each DoubleRow pair share one
    stationary tile, but bass emits an Ldweights per matmul and the
    compiler-side dedup (--enable-ldw-opt) is disabled.  Reloading
    identical weights is idempotent, so an Ldweights whose source AP
    equals the previous one on the PE stream — with only matmuls in
    between, no sync waits and no sem updates of its own — can be
    dropped.  W tiles are written once and never recycled, and x-tile
    slot reuse is gated on the matmuls' sem increments (Ldweights never
    increments), so sem bookkeeping is unchanged.
    """
    removed = 0
    for fn in nc.m.functions:
        for bb in fn.blocks:
            newl = []
            prev_ldw_key = None
            for inst in bb.instructions:
                t = type(inst).__name__
                eng = str(inst.engine)
                if eng == "EngineType.PE":
                    if t == "InstLdweights":
                        si = inst.sync_info
                        has_sync = si is not None and (si.on_wait or si.on_update)
                        key = str(inst.ins)
                        if key == prev_ldw_key and not has_sync:
                            removed += 1
                            continue
                        prev_ldw_key = key
                    elif t != "InstMatmult":
                        prev_ldw_key = None
                newl.append(inst)
            bb.instructions[:] = newl
    return removed


def _reduce_waits(nc):
    """Drop transitively-implied semaphore waits.

    A wait (sem, v) on instruction X is redundant when another wait on X
    targets a producer whose vector clock already covers (sem, v), when
    X's own proc has already observed it, or when the sem belongs to X's
    own in-order proc (same-FIFO dominance).  Two phases: build complete
    per-sem producer vector clocks (block list order is per-proc
    consistent; cross-proc misses only make the result conservative),
    then prune using the final maps.  Only 'sem-ge-imm' waits and
    incrementing ('sem-inc'/'sem-add-imm') updates participate; any other
    update mode invalidates that sem's history.
    """
    INC = ("sem-inc", "sem-add-imm")
    for fn in nc.m.functions:
        insts = [inst for bb in fn.blocks for inst in bb.instructions]

        def params(inst):
            si = inst.sync_info
            waits = list(si.on_wait) if si is not None and si.on_wait else []
            ups = list(si.on_update) if si is not None and si.on_update else []
            proc = (str(inst.engine), getattr(inst, "bass_scheduled_proc", None))
            return si, waits, ups, proc

        def wait_vc(prodvc, w):
            if w.wait_mode != "sem-ge-imm" or w.wait_reg is not None:
                return None
            for cv, vc in prodvc.get(w.id, []):
                if cv >= w.wait_value:
                    return vc
            return None

        def build_maps(lookup_prodvc):
            cum, prodvc, procvc, updaters, obsvc = {}, {}, {}, {}, {}
            for inst in insts:
                si, waits, ups, proc = params(inst)
                myvc = dict(procvc.get(proc, {}))
                for w in waits:
                    if w.wait_mode == "sem-ge-imm" and w.wait_reg is None:
                        vc = wait_vc(lookup_prodvc if lookup_prodvc is not None
                                     else prodvc, w)
                        if vc is not None:
                            for k, v in vc.items():
                                if myvc.get(k, 0) < v:
                                    myvc[k] = v
                        if myvc.get(w.id, 0) < w.wait_value:
                            myvc[w.id] = w.wait_value
                procvc[proc] = myvc
                obsvc[id(inst)] = myvc
                for u in ups:
                    if u.update_mode in INC and u.update_reg is None:
                        cum[u.id] = cum.get(u.id, 0) + u.update_value
                        updaters.setdefault(u.id, set()).add(proc)
                        snap = dict(myvc)
                        snap[u.id] = cum[u.id]
                        prodvc.setdefault(u.id, []).append((cum[u.id], snap))
                    else:
                        cum.pop(u.id, None)
                        prodvc.pop(u.id, None)
                        updaters[u.id] = {object()}
            return cum, prodvc, updaters, obsvc

        # Pass 1 builds conservative clocks; pass 2 rebuilds them resolving
        # forward references through pass 1's complete producer map.
        _, prodvc, _, _ = build_maps(None)
        _, prodvc, _, _ = build_maps(prodvc)

        # Prune with the final maps, tracking per-proc observation and
        # per-proc cumulative sem updates in list order.
        cum, procvc, updaters = {}, {}, {}
        for inst in insts:
            si, waits, ups, proc = params(inst)
            myvc = dict(procvc.get(proc, {}))
            if len(waits) > 1:
                vcs = [wait_vc(prodvc, w) for w in waits]
                keep_ws = []
                for i, w in enumerate(waits):
                    if w.wait_mode == "sem-ge-imm" and w.wait_reg is None:
                        if myvc.get(w.id, 0) >= w.wait_value:
                            continue
                        if (updaters.get(w.id) == {proc}
                                and cum.get(w.id, 0) >= w.wait_value):
                            continue
                        implied = any(
                            j != i and vcs[j] is not None
                            and vcs[j].get(w.id, 0) >= w.wait_value
                            for j in range(len(waits)))
                        if implied:
                            continue
                    keep_ws.append(w)
                if len(keep_ws) != len(waits):
                    inst.sync_info = mybir.SyncInfo(on_wait=keep_ws, on_update=ups)
                    waits = keep_ws
            for w in waits:
                if w.wait_mode == "sem-ge-imm" and w.wait_reg is None:
                    vc = wait_vc(prodvc, w)
                    if vc is not None:
                        for k, v in vc.items():
                            if myvc.get(k, 0) < v:
                                myvc[k] = v
                    if myvc.get(w.id, 0) < w.wait_value:
                        myvc[w.id] = w.wait_value
            procvc[proc] = myvc
            for u in ups:
                if u.update_mode in INC and u.update_reg is None:
                    cum[u.id] = cum.get(u.id, 0) + u.update_value
                    updaters.setdefault(u.id, set()).add(proc)
                else:
                    cum.pop(u.id, None)
                    updaters[u.id] = {object()}


def _legalize_waits(nc, keep=1, drain_cap=1):
    for fn in nc.m.functions:
        for bb in fn.blocks:
            newl = []
            for inst in bb.instructions:
                si = inst.sync_info
                waits = list(si.on_wait) if si is not None and si.on_wait else []
                if type(inst).__name__ in _WAIT_LIMITED and len(waits) > keep:
                    extra, kept = waits[:-keep], waits[-keep:]
                    for i in range(0, len(extra), drain_cap):
                        d = mybir.InstDrain(name=f"{inst.name}-wsplit{i}")
                        d.engine = inst.engine
                        d.sync_info = mybir.SyncInfo(
                            on_wait=extra[i : i + drain_cap], on_update=[])
                        newl.append(d)
                    inst.sync_info = mybir.SyncInfo(
                        on_wait=kept,
                        on_update=list(si.on_update) if si.on_update else [])
                newl.append(inst)
            bb.instructions[:] = newl
    return


_NC_CACHE = None


def build_nc(legalize=True):
    global _NC_CACHE
    if _NC_CACHE is not None:
        return _NC_CACHE
    nc = bass.Bass("TRN2", target_bir_lowering=False, debug=False)
    xt = nc.dram_tensor("xt", [NT, TOK_TILE // P, P, NG, P], f16, kind="ExternalInput").ap()
    w16 = nc.dram_tensor("w16", [P, NG, NW16], f16, kind="ExternalInput").ap()
    wf8 = nc.dram_tensor("wf8", [P, NPAIR, 2, NW8], f8, kind="ExternalInput").ap()
    biasv = nc.dram_tensor("biasv", [NSH], f32, kind="ExternalInput").ap()
    out = nc.dram_tensor("out", [M, NSH], f16, kind="ExternalOutput").ap()
    with tile.TileContext(nc) as tc:
        with ExitStack() as ctx:
            _kernel_body(ctx, tc, out, xt, w16, wf8, biasv)
    if legalize:
        _dedupe_ldweights(nc)
        _reduce_waits(nc)
        _legalize_waits(nc)
        _NC_CACHE = nc
    return nc


def prep_in_maps(inputs):
    """Host-side shard/layout prep.  Returns (in_maps, perm)."""
    x = np.asarray(inputs["x"], np.float32)
    hw = np.asarray(inputs["high_prec_weight"])
    hs = np.asarray(inputs["high_prec_scales"], np.float32)
    lw = np.asarray(inputs["low_prec_weight"])
    ls1 = np.asarray(inputs["low_prec_scales"], np.float32)
    ls2 = np.asarray(inputs["low_prec_scales2"], np.float32)
    lz = np.asarray(inputs["low_prec_zeros"], np.float32)
    perm = np.asarray(inputs["col_indices_inv"]).astype(np.int64)
    bias = np.asarray(inputs["bias"], np.float32)

    x2 = x.reshape(M, K)
    # x^T tiled per 128-token subtile:
    #   xt[t, s, p, g, m] = x[t*512 + s*128 + m, g*128 + p].
    xr = x2.reshape(NT, TOK_TILE // P, P, NG, P)
    xt = np.ascontiguousarray(xr.astype(np.float16).transpose(0, 1, 4, 3, 2))

    # Dequantized weights in concat order.
    w_high = (hw.astype(np.float32).reshape(NG, GROUP, N_HIGH)
              * hs[:, None, :]).reshape(K, N_HIGH)
    w_low = (lw.astype(np.float32) - lz) * ls1 * ls2

    bias_cat = np.empty(OUT_F, np.float32)
    bias_cat[perm] = bias  # bias_cat[col_inv[j]] = bias[j]

    in_maps = []
    for c in range(NCORES):
        hsl = slice(c * NH, (c + 1) * NH)
        lsl0 = c * NL            # stray 8 low cols -> fp16 path
        lsl8 = slice(c * NL + 8, (c + 1) * NL)  # 1024 low cols -> fp8 path
        w16c = np.concatenate(
            [w_high[:, hsl], w_low[:, lsl0:lsl0 + 8]], axis=1).astype(np.float16)
        w8c = w_low[:, lsl8].astype(ml_dtypes.float8_e4m3)
        biasv = np.concatenate(
            [bias_cat[c * NH:(c + 1) * NH],
             bias_cat[N_HIGH + c * NL:N_HIGH + (c + 1) * NL]]).astype(np.float32)
        in_maps.append({
            "xt": xt,
            "w16": np.ascontiguousarray(
                w16c.reshape(NG, P, NW16).transpose(1, 0, 2)),
            "wf8": np.ascontiguousarray(
                w8c.reshape(NPAIR, 2, P, NW8).transpose(2, 0, 1, 3)),
            "biasv": np.ascontiguousarray(biasv),
        })
    return in_maps, perm


def gather_output(results, perm):
    cat = np.empty((M, OUT_F), np.float32)
    for c in range(NCORES):
        o = results[c]["out"]
        cat[:, c * NH:(c + 1) * NH] = o[:, :NH]
        cat[:, N_HIGH + c * NL:N_HIGH + (c + 1) * NL] = o[:, NH:]
    final = np.take(cat, perm, axis=1)
    return np.ascontiguousarray(final.reshape(B, S, OUT_F).astype(np.float32))


def run(inputs, **spmd_kwargs):
    """Run on hardware; returns (output, BassKernelResults)."""
    nc = build_nc()
    in_maps, perm = prep_in_maps(inputs)
    res = run_bass_kernel_spmd(nc, in_maps, list(range(NCORES)), **spmd_kwargs)
    return gather_output(res.results, perm), res


def kernel(**inputs) -> np.ndarray:
    out, _ = run(inputs)
    return out

